# revision 1
# baseline (speedup 1.0000x reference)
"""Multi-head causal self-attention (B=4, S=2048, D=512, H=8) on 8 Trainium2
NeuronCores.

Sharding: core c handles batch b = c//2 and a 4-head group g = c%2
(heads 4g..4g+3, i.e. output-feature slice [256g, 256g+256)).  Each core's
output is a disjoint slice of the full output, so no collectives are needed.

Device kernel layout choices (per core):
  - inputs are passed transposed+bf16 (xT = x.T : [din, S]) so the
    projection matmuls can contract din on the partition dim.
  - Q,K are produced transposed ("QT/KT" = [dout, S]); attention scores are
    computed transposed: ST[k, q] = sum_d KT[d,k] * QT[d,q], which makes the
    softmax denominator and PV matmul contract over k on partitions.
  - softmax skips the max-subtraction: logits = s/8 with |s/8| <~ 6 for this
    problem's N(0,1)-ish inputs, safely inside exp's fp32 range.  exp runs on
    the scalar engine straight out of PSUM.
  - V is augmented with a ones-column, so the PV matmul accumulates both
    out^T[dv, q] and the softmax denominator (row 64) in one pass.
  - normalization (divide by denom) + final transpose happen on the host
    during the gather step.
"""

import numpy as np
import ml_dtypes

from concourse import bacc, mybir
from concourse.tile import TileContext
from concourse.bass_utils import run_bass_kernel_spmd

BF16 = mybir.dt.bfloat16
F32 = mybir.dt.float32
AF = mybir.ActivationFunctionType
BFNP = ml_dtypes.bfloat16

B, S, D = 4, 2048, 512
H, HD = 8, 64
HPC = 4                   # heads per core
DSL = HPC * HD            # 256-wide output-feature slice per core
N_CORES = 8
SCALE = float(HD) ** 0.5  # 8.0
QH_W = 1024               # q processed in two halves of 1024


# timing instrumentation only: emit the compute body N times (identical
# output; wall-clock delta between variants isolates device compute time)
PASSES = 1


def build_nc():
    nc = bacc.Bacc("TRN2", target_bir_lowering=False)

    qT = nc.declare_dram_parameter("qT", [D, S], BF16, isOutput=False)
    kTd = nc.declare_dram_parameter("kTd", [D, S], BF16, isOutput=False)
    vT = nc.declare_dram_parameter("vT", [D, S], BF16, isOutput=False)
    wqT = nc.declare_dram_parameter("wqT", [D, DSL], BF16, isOutput=False)
    wkT = nc.declare_dram_parameter("wkT", [D, DSL], BF16, isOutput=False)
    wvT = nc.declare_dram_parameter("wvT", [D, DSL], BF16, isOutput=False)
    # packed small tensors: [0:2]=bq, [2:4]=bk, [4:260]=bvb, [260:324]=mask(bf16 bits)
    smallp = nc.declare_dram_parameter("smallp", [128, 324], F32, isOutput=False)
    # rows [65h, 65h+64) = unnormalized out^T for head h; row 65h+64 = denom
    out_t = nc.declare_dram_parameter(
        "out_t", [HPC * (HD + 1), S], F32, isOutput=True
    )

    with TileContext(nc) as tc:
        with tc.tile_pool(name="const", bufs=1) as cpool:
            qT_sb = cpool.tile([128, 4, S], BF16, tag="qT_sb")
            kT_sb = cpool.tile([128, 4, S], BF16, tag="kT_sb")
            vT_sb = cpool.tile([128, 4, S], BF16, tag="vT_sb")
            wq_sb = cpool.tile([128, 4, DSL], BF16, tag="wq_sb")
            wk_sb = cpool.tile([128, 4, DSL], BF16, tag="wk_sb")
            wv_sb = cpool.tile([128, 4, DSL], BF16, tag="wv_sb")
            small_sb = cpool.tile([128, 324], F32, tag="small_sb")
            bq_sb = small_sb[:, 0:2]
            bk_sb = small_sb[:, 2:4]
            bvb_sb = small_sb[:, 4:260]
            mask_sb = small_sb[:, 260:324].bitcast(BF16)
            # projected tensors: chunk dim = head pair (dout 128-chunk)
            QT_sb = cpool.tile([128, 2, S], BF16, tag="QT_sb")
            KT_sb = cpool.tile([128, 2, S], BF16, tag="KT_sb")
            # V with ones column: [k-part, head, k-tile, dv+1]
            vaug_sb = cpool.tile([128, HPC, 16, HD + 1], BF16, tag="vaug_sb")

            # only the ones-column needs init; cols 0..63 are written by proj_v
            nc.vector.memset(vaug_sb[:, :, :, HD : HD + 1], 1.0)

            def load_w(w_sb, wsrc, eng):
                eng.dma_start(w_sb[:], wsrc[:].rearrange("(c p) m -> p c m", p=128))

            def load_x(dstt, srcd, sq, eng):
                s0 = 512 * sq
                eng.dma_start(
                    dstt[:, :, s0 : s0 + 512],
                    srcd[:, s0 : s0 + 512].rearrange("(c p) s -> p c s", p=128),
                )

            # loads ordered by when the first attention tiles need them
            _Q, _K, _V = (qT_sb, qT), (kT_sb, kTd), (vT_sb, vT)
            load_w(wv_sb, wvT, nc.sync)
            load_x(*_V, 0, nc.sync)
            load_w(wk_sb, wkT, nc.sync)
            load_x(*_K, 0, nc.sync)
            nc.sync.dma_start(small_sb[:], smallp[:])
            load_w(wq_sb, wqT, nc.sync)
            load_x(*_Q, 0, nc.sync)
            load_x(*_Q, 1, nc.sync)
            for xt, sq in ((_K, 1), (_V, 1), (_Q, 2), (_Q, 3),
                           (_K, 2), (_K, 3), (_V, 2), (_V, 3)):
                load_x(*xt, sq, nc.sync)

            # ---- projections + attention, interleaved ----
            # PSUM budget: ppool 2x1 + spool 2x2 + apool 1x2 = 8 banks
            with (
                tc.tile_pool(name="ppsum", bufs=2, space="PSUM") as ppool,
                tc.tile_pool(name="spsum", bufs=2, space="PSUM") as spool,
                tc.tile_pool(name="apsum", bufs=1, space="PSUM") as apool,
                tc.tile_pool(name="epool", bufs=7) as epool,
                tc.tile_pool(name="opool", bufs=3) as opool,
            ):

                def proj_v_st(st):
                    ps = ppool.tile([128, 512], F32, tag="pproj", name="psv")
                    for dc in range(4):
                        nc.tensor.matmul(
                            ps[:, 0:DSL],
                            vT_sb[:, dc, 128 * st : 128 * st + 128],
                            wv_sb[:, dc, :],
                            start=(dc == 0),
                            stop=(dc == 3),
                        )
                    for hh in range(HPC):
                        nc.vector.tensor_add(
                            vaug_sb[:, hh, st, 0:HD],
                            ps[:, HD * hh : HD * hh + HD],
                            bvb_sb[:, HD * hh : HD * hh + HD],
                        )

                QSRC = (wq_sb, bq_sb, qT_sb, QT_sb)
                KSRC = (wk_sb, bk_sb, kT_sb, KT_sb)

                def proj_qk_tile(mc, sc, src):
                    w_sb, b_sb, x_sb, dst = src
                    ps = ppool.tile([128, 512], F32, tag="pproj", name="psqk")
                    for dc in range(4):
                        nc.tensor.matmul(
                            ps[:],
                            w_sb[:, dc, 128 * mc : 128 * mc + 128],
                            x_sb[:, dc, 512 * sc : 512 * sc + 512],
                            start=(dc == 0),
                            stop=(dc == 3),
                        )
                    nc.vector.tensor_scalar_add(
                        dst[:, mc, 512 * sc : 512 * sc + 512],
                        ps[:],
                        b_sb[:, mc : mc + 1],
                    )

                def attn_head(h, sched=None):
                    sched = sched or {}
                    mc, prow = h // 2, 64 * (h % 2)
                    GROUPS = {
                        0: [(0,), (1,), (2,), (3,), (4, 5), (6, 7)],
                        1: [(k,) for k in range(12)] + [(12, 13), (14, 15)],
                    }

                    def geom(qh, kt):
                        Q0 = QH_W * qh
                        K0 = 128 * kt
                        qlo = max(Q0, K0)
                        return K0, qlo, Q0 + QH_W - qlo

                    def grp_offsets(qh, grp):
                        # pack members tightly; a scores region must not
                        # cross a 512-element PSUM bank boundary
                        pos, offs = 0, []
                        for kt in grp:
                            W = geom(qh, kt)[2]
                            if pos % 512 + min(W, 512) > 512:
                                pos = (pos + 511) // 512 * 512
                            offs.append(pos)
                            pos += W
                        return offs, pos

                    def scores_grp(qh, gi):
                        sl = spool.tile([128, QH_W], F32, tag="sl", name="sl")
                        offs = grp_offsets(qh, GROUPS[qh][gi])[0]
                        for j, kt in enumerate(GROUPS[qh][gi]):
                            K0, qlo, W = geom(qh, kt)
                            base = offs[j]
                            for c0 in range(0, W, 512):
                                cw = min(512, W - c0)
                                nc.tensor.matmul(
                                    sl[:, base + c0 : base + c0 + cw],
                                    KT_sb[prow : prow + 64, mc, K0 : K0 + 128],
                                    QT_sb[
                                        prow : prow + 64, mc,
                                        qlo + c0 : qlo + c0 + cw,
                                    ],
                                    start=True,
                                    stop=True,
                                )
                        return sl

                    hoisted = None
                    for qh in range(2):
                        Q0 = QH_W * qh
                        kmax = 8 if qh == 0 else 16
                        groups = GROUPS[qh]
                        acc = apool.tile([HD + 1, QH_W], F32, tag="acc", name="acc")
                        # software pipeline: scores run one group ahead of PV
                        sl = hoisted if hoisted is not None else scores_grp(qh, 0)
                        hoisted = None
                        for gi, grp in enumerate(groups):
                            goffs, We = grp_offsets(qh, grp)
                            et = epool.tile([128, QH_W], BF16, tag="et", name="et")
                            nc.scalar.activation(
                                et[:, 0:We], sl[:, 0:We], AF.Exp, scale=1.0 / SCALE
                            )
                            if gi + 1 < len(groups):
                                sl = scores_grp(qh, gi + 1)
                            elif qh == 0:
                                # hoist next q-half's first scores ahead of
                                # this group's trailing PV matmuls
                                hoisted = scores_grp(1, 0)
                            for kt in grp:
                                if h == 0 and (qh == 0 or kt >= 8) and kt >= 4:
                                    proj_v_st(kt)  # st == kt; fills vaug for PV
                                for work in sched.get((qh, kt), ()):
                                    work()  # deferred projection tile
                            for j, kt in enumerate(grp):
                                K0, qlo, W = geom(qh, kt)
                                off = qlo - Q0
                                base = goffs[j]
                                if K0 >= Q0:
                                    nc.vector.tensor_mul(
                                        et[:, base : base + 128],
                                        et[:, base : base + 128],
                                        mask_sb[:],
                                    )
                                b0 = off
                                while b0 < QH_W:
                                    b1 = min(QH_W, (b0 // 512 + 1) * 512)
                                    nc.tensor.matmul(
                                        acc[:, b0:b1],
                                        vaug_sb[:, h, kt, :],
                                        et[:, base + b0 - off : base + b1 - off],
                                        start=(kt == 0),
                                        stop=(kt == kmax - 1),
                                        skip_group_check=True,
                                    )
                                    b0 = b1
                        ot = opool.tile([HD + 1, QH_W], F32, tag="ot", name="ot")
                        nc.vector.tensor_copy(ot[:], acc[:])
                        nc.sync.dma_start(
                            out_t[(HD + 1) * h : (HD + 1) * h + HD + 1, Q0 : Q0 + QH_W],
                            ot[:],
                        )

                def qk_tile(mc, sc, s):
                    return lambda: proj_qk_tile(mc, sc, s)

                for _pass in range(PASSES):
                    # prologue: only the tiles the first scores/PV need, V-proj
                    # interleaved to fill DMA-wait bubbles
                    proj_v_st(0)
                    proj_v_st(1)
                    proj_qk_tile(0, 0, KSRC)
                    proj_qk_tile(0, 0, QSRC)
                    proj_qk_tile(0, 1, QSRC)
                    proj_v_st(2)
                    proj_v_st(3)
                    q1 = [qk_tile(1, sc, s) for sc, s in (
                        (0, KSRC), (0, QSRC), (1, QSRC), (1, KSRC),
                        (2, QSRC), (2, KSRC), (3, QSRC), (3, KSRC))]
                    # deferred tiles, placed just before their deadlines in
                    # windows where ACT (exp) is the busier engine
                    attn_head(0, sched={
                        (0, 0): [qk_tile(0, 1, KSRC)],
                        (0, 1): [qk_tile(0, 2, QSRC)],
                        (0, 2): [qk_tile(0, 3, QSRC)],
                        (1, 0): [qk_tile(0, 2, KSRC)],
                        (1, 1): [qk_tile(0, 3, KSRC)],
                        (1, 2): [q1[0]], (1, 3): [q1[1]], (1, 4): [q1[2]],
                        (1, 5): [q1[3]], (1, 6): [q1[4]], (1, 7): [q1[5]],
                    })
                    attn_head(1, sched={(1, 0): [q1[6]], (1, 1): [q1[7]]})
                    attn_head(2)
                    attn_head(3)

    nc.finalize()
    return nc


_NC_CACHE = {}


def _get_nc():
    if "nc" not in _NC_CACHE:
        _NC_CACHE["nc"] = build_nc()
    return _NC_CACHE["nc"]


def make_in_maps(query, key, value, Wq, bq, Wk, bk, Wv, bv):
    query, key, value = (np.asarray(x, np.float32) for x in (query, key, value))
    Wq, Wk, Wv = (np.asarray(x, np.float32) for x in (Wq, Wk, Wv))
    bq, bk, bv = (np.asarray(x, np.float32) for x in (bq, bk, bv))
    mask = np.triu(np.ones((128, 128), np.float32)).astype(BFNP)

    def pack_small(bqs, bks, bvs, m):
        out = np.empty((128, 324), np.float32)
        out[:, 0:2] = bqs.reshape(2, 128).T
        out[:, 2:4] = bks.reshape(2, 128).T
        out[:, 4:260] = np.tile(bvs[None, :], (128, 1))
        out[:, 260:324] = np.ascontiguousarray(m).view(np.float32)
        return out

    in_maps = []
    for c in range(N_CORES):
        b, g = c // 2, c % 2
        sl = slice(DSL * g, DSL * g + DSL)
        in_maps.append(
            {
                "qT": np.ascontiguousarray(query[b].astype(BFNP).T),
                "kTd": np.ascontiguousarray(key[b].astype(BFNP).T),
                "vT": np.ascontiguousarray(value[b].astype(BFNP).T),
                "wqT": np.ascontiguousarray(Wq[sl].astype(BFNP).T),
                "wkT": np.ascontiguousarray(Wk[sl].astype(BFNP).T),
                "wvT": np.ascontiguousarray(Wv[sl].astype(BFNP).T),
                "smallp": pack_small(bq[sl], bk[sl], bv[sl], mask),
            }
        )
    return in_maps


def assemble_output(results):
    out = np.empty((B, S, D), np.float32)
    for c in range(N_CORES):
        b, g = c // 2, c % 2
        ot = results[c]["out_t"]  # [260, 2048]
        for hl in range(HPC):
            blk = ot[(HD + 1) * hl : (HD + 1) * hl + HD]  # [64, S]
            den = ot[(HD + 1) * hl + HD]  # [S]
            h = HPC * g + hl
            out[b, :, HD * h : HD * h + HD] = (blk / den).T
    return out


def run(trace=False, **inputs):
    nc = _get_nc()
    in_maps = make_in_maps(**inputs)
    res = run_bass_kernel_spmd(nc, in_maps, list(range(N_CORES)), trace=trace)
    return assemble_output(res.results), res


def kernel(**inputs) -> np.ndarray:
    out, _ = run(trace=False, **inputs)
    return out



# revision 2
# speedup vs baseline: 1.1108x; 1.1108x over previous
"""Multi-head causal self-attention (B=4, S=2048, D=512, H=8) on 8 Trainium2
NeuronCores.

Sharding: core c handles batch b = c//2 and a 4-head group g = c%2
(heads 4g..4g+3, i.e. output-feature slice [256g, 256g+256)).  Each core's
output is a disjoint slice of the full output, so no collectives are needed.

Device kernel layout (per core), v2 — flat software pipeline:
  - inputs are passed transposed+bf16 (xT = x.T : [din, S]) so the
    projection matmuls can contract din on the partition dim.
  - Q,K are produced transposed ("QT/KT" = [dout, S]); attention scores are
    computed transposed: ST[k, q] = sum_d KT[d,k] * QT[d,q].
  - softmax skips the max-subtraction (|s/8| <~ 6 for N(0,1)-ish inputs);
    exp runs on the scalar engine straight out of PSUM.
  - V is augmented with a ones-column so the PV matmul also accumulates the
    softmax denominator; normalization + transpose happen on the host.
  - the PE and ACT engines process attention columns at the same rate
    (0.833 ns/col), so the schedule is a single flat pipeline over all
    (head, q-half) pairs with one-group scores lookahead; projection tiles
    are placed as PE filler exactly where the ACT per-instruction overhead
    would otherwise stall the PE stream.
"""

import numpy as np
import ml_dtypes

from concourse import bacc, mybir
from concourse.tile import TileContext
from concourse.bass_utils import run_bass_kernel_spmd

BF16 = mybir.dt.bfloat16
F32 = mybir.dt.float32
AF = mybir.ActivationFunctionType
BFNP = ml_dtypes.bfloat16

B, S, D = 4, 2048, 512
H, HD = 8, 64
HPC = 4                   # heads per core
DSL = HPC * HD            # 256-wide output-feature slice per core
N_CORES = 8
SCALE = float(HD) ** 0.5  # 8.0
QH_W = 1024               # q processed in two halves of 1024


# timing instrumentation only: emit the compute body N times (identical
# output; wall-clock delta between variants isolates device compute time)
PASSES = 1


def std_groups(qh):
    """Standard score groups for a q-half: list of groups; each group is a
    list of (kt, a, b) column ranges (absolute q coords) of k-tile kt."""
    Q0 = QH_W * qh
    kmax = 8 if qh == 0 else 16

    def ent(kt):
        K0 = 128 * kt
        return (kt, max(Q0, K0), Q0 + QH_W)

    if qh == 0:
        idx = [(0,), (1,), (2,), (3,), (4, 5), (6, 7)]
    else:
        idx = [(k,) for k in range(12)] + [(12, 13), (14, 15)]
    return [[ent(kt) for kt in g] for g in idx]


def h0q0_groups():
    """First half: split so the first scores only need Q[:, 0:512]."""
    return [
        [(0, 0, 512)],
        [(1, 128, 512), (2, 256, 512), (3, 384, 512)],
        [(0, 512, 1024)],
        [(1, 512, 1024), (2, 512, 1024)],
        [(3, 512, 1024), (4, 512, 1024)],
        [(5, 640, 1024), (6, 768, 1024), (7, 896, 1024)],
    ]


def build_nc():
    nc = bacc.Bacc("TRN2", target_bir_lowering=False)

    qT = nc.declare_dram_parameter("qT", [D, S], BF16, isOutput=False)
    kTd = nc.declare_dram_parameter("kTd", [D, S], BF16, isOutput=False)
    vT = nc.declare_dram_parameter("vT", [D, S], BF16, isOutput=False)
    wqT = nc.declare_dram_parameter("wqT", [D, DSL], BF16, isOutput=False)
    wkT = nc.declare_dram_parameter("wkT", [D, DSL], BF16, isOutput=False)
    wvT = nc.declare_dram_parameter("wvT", [D, DSL], BF16, isOutput=False)
    # packed small tensors: [0:2]=bq, [2:4]=bk, [4:260]=bvb, [260:324]=mask(bf16 bits)
    smallp = nc.declare_dram_parameter("smallp", [128, 324], F32, isOutput=False)
    # rows [65h, 65h+64) = unnormalized out^T for head h; row 65h+64 = denom
    out_t = nc.declare_dram_parameter(
        "out_t", [HPC * (HD + 1), S], F32, isOutput=True
    )

    with TileContext(nc) as tc:
        with tc.tile_pool(name="const", bufs=1) as cpool:
            qT_sb = cpool.tile([128, 4, S], BF16, tag="qT_sb")
            kT_sb = cpool.tile([128, 4, S], BF16, tag="kT_sb")
            vT_sb = cpool.tile([128, 4, S], BF16, tag="vT_sb")
            wq_sb = cpool.tile([128, 4, DSL], BF16, tag="wq_sb")
            wk_sb = cpool.tile([128, 4, DSL], BF16, tag="wk_sb")
            wv_sb = cpool.tile([128, 4, DSL], BF16, tag="wv_sb")
            small_sb = cpool.tile([128, 324], F32, tag="small_sb")
            bq_sb = small_sb[:, 0:2]
            bk_sb = small_sb[:, 2:4]
            bvb_sb = small_sb[:, 4:260]
            mask_sb = small_sb[:, 260:324].bitcast(BF16)
            # projected tensors: chunk dim = head pair (dout 128-chunk)
            QT_sb = cpool.tile([128, 2, S], BF16, tag="QT_sb")
            KT_sb = cpool.tile([128, 2, S], BF16, tag="KT_sb")
            # V with ones column: [k-part, head, k-tile, dv+1]
            vaug_sb = cpool.tile([128, HPC, 16, HD + 1], BF16, tag="vaug_sb")

            # only the ones-column needs init; cols 0..63 are written by proj_v
            nc.vector.memset(vaug_sb[:, :, :, HD : HD + 1], 1.0)

            def load_w_dc(w_sb, wsrc, dc0, dc1):
                if dc1 - dc0 == 1:
                    nc.sync.dma_start(
                        w_sb[:, dc0, :], wsrc[128 * dc0 : 128 * dc0 + 128, :]
                    )
                else:
                    nc.sync.dma_start(
                        w_sb[:, dc0:dc1, :],
                        wsrc[128 * dc0 : 128 * dc1, :].rearrange(
                            "(c p) m -> p c m", p=128
                        ),
                    )

            def load_x(dstt, srcd, s0, s1):
                nc.sync.dma_start(
                    dstt[:, :, s0:s1],
                    srcd[:, s0:s1].rearrange("(c p) s -> p c s", p=128),
                )

            # DMA order: prioritize the K->Q scores chain, V next, big tails
            # last.  First K chunk is dc-split so the PE can start ~2us in.
            nc.sync.dma_start(wk_sb[:, 0, :], wkT[0:128, :])
            nc.sync.dma_start(kT_sb[:, 0, 0:512], kTd[0:128, 0:512])
            load_w_dc(wk_sb, wkT, 1, 4)
            nc.sync.dma_start(
                kT_sb[:, 1:4, 0:512],
                kTd[128:512, 0:512].rearrange("(c p) s -> p c s", p=128),
            )
            nc.sync.dma_start(small_sb[:], smallp[:])
            load_w_dc(wq_sb, wqT, 0, 4)
            load_x(qT_sb, qT, 0, 512)
            load_x(qT_sb, qT, 512, 1024)
            load_x(kT_sb, kTd, 512, 1024)
            load_w_dc(wv_sb, wvT, 0, 4)
            load_x(vT_sb, vT, 0, 512)
            load_x(vT_sb, vT, 512, 1024)
            load_x(qT_sb, qT, 1024, 2048)
            load_x(kT_sb, kTd, 1024, 2048)
            load_x(vT_sb, vT, 1024, 2048)

            # ---- projections + attention, flat pipeline ----
            # PSUM budget: ppool 2x1 + spool 2x2 + apool 1x2 = 8 banks
            with (
                tc.tile_pool(name="ppsum", bufs=2, space="PSUM") as ppool,
                tc.tile_pool(name="spsum", bufs=2, space="PSUM") as spool,
                tc.tile_pool(name="apsum", bufs=1, space="PSUM") as apool,
                tc.tile_pool(name="epool", bufs=8) as epool,
                tc.tile_pool(name="opool", bufs=4) as opool,
            ):

                def proj_v(st, h0, nh):
                    """V projection for seq window st, heads [h0, h0+nh)."""
                    c0, w = HD * h0, HD * nh
                    ps = ppool.tile([128, 512], F32, tag="pproj", name="psv")
                    for dc in range(4):
                        nc.tensor.matmul(
                            ps[:, 0:w],
                            vT_sb[:, dc, 128 * st : 128 * st + 128],
                            wv_sb[:, dc, c0 : c0 + w],
                            start=(dc == 0),
                            stop=(dc == 3),
                        )
                    nc.vector.tensor_add(
                        vaug_sb[:, h0 : h0 + nh, st, 0:HD],
                        ps[:, 0:w].rearrange("p (h d) -> p h d", h=nh),
                        bvb_sb[:, c0 : c0 + w].rearrange("p (h d) -> p h d", h=nh),
                    )

                QSRC = (wq_sb, bq_sb, qT_sb, QT_sb)
                KSRC = (wk_sb, bk_sb, kT_sb, KT_sb)

                def proj_qk_tile(mc, sc, src):
                    w_sb, b_sb, x_sb, dst = src
                    ps = ppool.tile([128, 512], F32, tag="pproj", name="psqk")
                    for dc in range(4):
                        nc.tensor.matmul(
                            ps[:],
                            w_sb[:, dc, 128 * mc : 128 * mc + 128],
                            x_sb[:, dc, 512 * sc : 512 * sc + 512],
                            start=(dc == 0),
                            stop=(dc == 3),
                        )
                    nc.vector.tensor_scalar_add(
                        dst[:, mc, 512 * sc : 512 * sc + 512],
                        ps[:],
                        b_sb[:, mc : mc + 1],
                    )

                def grp_offsets(grp):
                    # pack members tightly; a scores region must not
                    # cross a 512-element PSUM bank boundary
                    pos, offs = 0, []
                    for kt, a, b in grp:
                        W = b - a
                        if pos % 512 + min(W, 512) > 512:
                            pos = (pos + 511) // 512 * 512
                        offs.append(pos)
                        pos += W
                    return offs, pos

                def scores_grp(h, grp):
                    mc, prow = h // 2, 64 * (h % 2)
                    sl = spool.tile([128, QH_W], F32, tag="sl", name="sl")
                    offs = grp_offsets(grp)[0]
                    for (kt, a, b), base in zip(grp, offs):
                        K0, W = 128 * kt, b - a
                        for c0 in range(0, W, 512):
                            cw = min(512, W - c0)
                            nc.tensor.matmul(
                                sl[:, base + c0 : base + c0 + cw],
                                KT_sb[prow : prow + 64, mc, K0 : K0 + 128],
                                QT_sb[prow : prow + 64, mc, a + c0 : a + c0 + cw],
                                start=True,
                                stop=True,
                            )
                    return sl

                # ---- flat schedule over (head, q-half) pairs ----
                HALVES = [(0, 0), (1, 0), (0, 1), (1, 1),
                          (2, 0), (3, 0), (2, 1), (3, 1)]

                def qk(mc, sc, s):
                    return lambda: proj_qk_tile(mc, sc, s)

                def pv(st, h0, nh):
                    return lambda: proj_v(st, h0, nh)

                def make_scheds():
                    # per-half dict: group index -> ("slot", [thunks]) where
                    # slot M = after the scores lookahead, before this
                    # group's PV (PE filler while ACT runs exp); slot T =
                    # after this group's PV (for tiles whose input DMA lands
                    # later — keeps them off the PV critical path)
                    M, T = "M", "T"
                    return {
                        (0, 0): {
                            0: [(M, qk(0, 1, QSRC)), (T, qk(1, 0, QSRC))],
                            2: [(M, qk(0, 1, KSRC))],
                            3: [(M, qk(1, 1, QSRC))],
                            4: [(M, pv(0, 0, 2)), (M, pv(1, 0, 2)),
                                (M, pv(2, 0, 2)), (M, pv(3, 0, 2))],
                            5: [(M, pv(4, 0, 2)), (M, pv(5, 0, 2)),
                                (M, qk(1, 1, KSRC))],
                        },
                        (1, 0): {
                            0: [(M, pv(6, 0, 2)), (M, pv(7, 0, 2))],
                            2: [(M, qk(0, 2, QSRC))],
                            4: [(M, qk(0, 3, QSRC))],
                        },
                        (0, 1): {
                            0: [(M, qk(1, 2, QSRC))],
                            2: [(M, qk(1, 3, QSRC))],
                            5: [(M, qk(0, 2, KSRC))],
                            7: [(M, pv(8, 0, 2))],
                            8: [(M, pv(9, 0, 2))],
                            9: [(M, qk(0, 3, KSRC)), (M, pv(10, 0, 2))],
                            10: [(M, pv(11, 0, 2))],
                            11: [(M, pv(12, 0, 2))],
                            12: [(M, pv(13, 0, 2)), (M, pv(14, 0, 2))],
                            13: [(M, pv(15, 0, 2))],
                        },
                        (1, 1): {
                            0: [(M, qk(1, 2, KSRC))],
                            4: [(M, qk(1, 3, KSRC))],
                        },
                        (2, 0): {
                            0: [(M, pv(0, 2, 2))],
                            1: [(M, pv(1, 2, 2)), (M, pv(2, 2, 2))],
                            2: [(M, pv(3, 2, 2))],
                            3: [(M, pv(4, 2, 2))],
                            4: [(M, pv(5, 2, 2))],
                            5: [(M, pv(6, 2, 2)), (M, pv(7, 2, 2))],
                        },
                        (3, 0): {
                            0: [(M, pv(8, 2, 1))],
                            1: [(M, pv(9, 2, 1)), (M, pv(10, 2, 1))],
                            2: [(M, pv(11, 2, 1)), (M, pv(8, 3, 1))],
                            3: [(M, pv(9, 3, 1)), (M, pv(10, 3, 1))],
                            4: [(M, pv(11, 3, 1))],
                        },
                        (2, 1): {
                            8: [(M, pv(12, 2, 1))],
                            9: [(M, pv(13, 2, 1))],
                            10: [(M, pv(14, 2, 1))],
                            11: [(M, pv(15, 2, 1))],
                        },
                        (3, 1): {
                            8: [(M, pv(12, 3, 1))],
                            9: [(M, pv(13, 3, 1))],
                            10: [(M, pv(14, 3, 1))],
                            11: [(M, pv(15, 3, 1))],
                        },
                    }

                def flush(h, qh, acc, c0, c1):
                    ot = opool.tile([HD + 1, QH_W], F32, tag="ot", name="ot")
                    nc.vector.tensor_copy(ot[:, c0:c1], acc[:, c0:c1])
                    r0 = (HD + 1) * h
                    Q0 = QH_W * qh
                    nc.sync.dma_start(
                        out_t[r0 : r0 + HD + 1, Q0 + c0 : Q0 + c1],
                        ot[:, c0:c1],
                    )

                def emit_pv(rec):
                    # PV matmuls for a group, one pipeline stage behind the
                    # exp: everything here is ready, so the PE never blocks
                    h, qh, grp, goffs, et, acc, post = rec
                    Q0 = QH_W * qh
                    kmax = 8 if qh == 0 else 16
                    for (kt, a, b), base in zip(grp, goffs):
                        off = a - Q0
                        b0 = off
                        while b0 < off + (b - a):
                            b1 = min(off + (b - a), (b0 // 512 + 1) * 512)
                            nc.tensor.matmul(
                                acc[:, b0:b1],
                                vaug_sb[:, h, kt, :],
                                et[:, base + b0 - off : base + b1 - off],
                                start=(kt == 0),
                                stop=(kt == kmax - 1),
                                skip_group_check=True,
                            )
                            b0 = b1
                    for fl in post:
                        fl()

                for _pass in range(PASSES):
                    scheds = make_scheds()
                    groups_of = {
                        hq: (h0q0_groups() if hq == (0, 0) else std_groups(hq[1]))
                        for hq in HALVES
                    }
                    # prologue: projections feeding the very first scores;
                    # K(mc1) last so its bias-add doesn't gate the Q tile
                    # through the ppool rotation
                    proj_qk_tile(0, 0, KSRC)
                    proj_qk_tile(0, 0, QSRC)
                    proj_qk_tile(1, 0, KSRC)

                    pending_sl = scores_grp(HALVES[0][0], groups_of[HALVES[0]][0])
                    pvq = []  # deferred PV records (deeper lag at startup
                    #           so early PVs don't block the PE stream while
                    #           the V DMAs are still in flight)
                    for hi, (h, qh) in enumerate(HALVES):
                        groups = groups_of[(h, qh)]
                        sched = scheds[(h, qh)]
                        acc = apool.tile([HD + 1, QH_W], F32, tag="acc",
                                         name="acc")
                        last_half = hi + 1 >= len(HALVES)
                        for gi, grp in enumerate(groups):
                            sl = pending_sl
                            goffs, We = grp_offsets(grp)
                            et = epool.tile([128, QH_W], BF16, tag="et",
                                            name="et")
                            nc.scalar.activation(
                                et[:, 0:We], sl[:, 0:We], AF.Exp,
                                scale=1.0 / SCALE,
                            )
                            for (kt, a, b), base in zip(grp, goffs):
                                if a == 128 * kt:  # diagonal: causal mask
                                    nc.vector.tensor_mul(
                                        et[:, base : base + 128],
                                        et[:, base : base + 128],
                                        mask_sb[:],
                                    )
                            # one-group scores lookahead (crosses halves)
                            if gi + 1 < len(groups):
                                pending_sl = scores_grp(h, groups[gi + 1])
                            elif not last_half:
                                nh, nqh = HALVES[hi + 1]
                                pending_sl = scores_grp(
                                    nh, groups_of[(nh, nqh)][0]
                                )
                            else:
                                pending_sl = None
                            for slot, work in sched.get(gi, ()):
                                if slot == "M":
                                    work()
                            lag = 99 if (hi == 0 and gi < 4) else 1
                            while len(pvq) > lag:
                                emit_pv(pvq.pop(0))
                            for slot, work in sched.get(gi, ()):
                                if slot == "T":
                                    work()
                            # flushes ride on the PV record (run after it);
                            # the final half streams out in 4 chunks, each as
                            # soon as its last k-tile has accumulated
                            post = []

                            def fl(c0, c1, h_=h, q_=qh, a_=acc):
                                return lambda: flush(h_, q_, a_, c0, c1)

                            if last_half and gi == 11:
                                post.append(fl(0, 512))
                            if last_half and gi == 12:
                                post.append(fl(512, 768))
                            if gi + 1 == len(groups):
                                if last_half:
                                    post.append(fl(768, QH_W))
                                else:
                                    post.append(fl(0, QH_W))
                            pvq.append((h, qh, grp, goffs, et, acc, post))
                    # drain the final PVs after the last exp
                    for rec in pvq:
                        emit_pv(rec)

    nc.finalize()
    return nc


_NC_CACHE = {}


def _get_nc():
    if "nc" not in _NC_CACHE:
        _NC_CACHE["nc"] = build_nc()
    return _NC_CACHE["nc"]


def make_in_maps(query, key, value, Wq, bq, Wk, bk, Wv, bv):
    query, key, value = (np.asarray(x, np.float32) for x in (query, key, value))
    Wq, Wk, Wv = (np.asarray(x, np.float32) for x in (Wq, Wk, Wv))
    bq, bk, bv = (np.asarray(x, np.float32) for x in (bq, bk, bv))
    mask = np.triu(np.ones((128, 128), np.float32)).astype(BFNP)

    def pack_small(bqs, bks, bvs, m):
        out = np.empty((128, 324), np.float32)
        out[:, 0:2] = bqs.reshape(2, 128).T
        out[:, 2:4] = bks.reshape(2, 128).T
        out[:, 4:260] = np.tile(bvs[None, :], (128, 1))
        out[:, 260:324] = np.ascontiguousarray(m).view(np.float32)
        return out

    in_maps = []
    for c in range(N_CORES):
        b, g = c // 2, c % 2
        sl = slice(DSL * g, DSL * g + DSL)
        in_maps.append(
            {
                "qT": np.ascontiguousarray(query[b].astype(BFNP).T),
                "kTd": np.ascontiguousarray(key[b].astype(BFNP).T),
                "vT": np.ascontiguousarray(value[b].astype(BFNP).T),
                "wqT": np.ascontiguousarray(Wq[sl].astype(BFNP).T),
                "wkT": np.ascontiguousarray(Wk[sl].astype(BFNP).T),
                "wvT": np.ascontiguousarray(Wv[sl].astype(BFNP).T),
                "smallp": pack_small(bq[sl], bk[sl], bv[sl], mask),
            }
        )
    return in_maps


def assemble_output(results):
    out = np.empty((B, S, D), np.float32)
    for c in range(N_CORES):
        b, g = c // 2, c % 2
        ot = results[c]["out_t"]  # [260, 2048]
        for hl in range(HPC):
            blk = ot[(HD + 1) * hl : (HD + 1) * hl + HD]  # [64, S]
            den = ot[(HD + 1) * hl + HD]  # [S]
            h = HPC * g + hl
            out[b, :, HD * h : HD * h + HD] = (blk / den).T
    return out


def run(trace=False, **inputs):
    nc = _get_nc()
    in_maps = make_in_maps(**inputs)
    res = run_bass_kernel_spmd(nc, in_maps, list(range(N_CORES)), trace=trace)
    return assemble_output(res.results), res


def kernel(**inputs) -> np.ndarray:
    out, _ = run(trace=False, **inputs)
    return out


# revision 8
# speedup vs baseline: 1.1752x; 1.0580x over previous
"""Multi-head causal self-attention (B=4, S=2048, D=512, H=8) on 8 Trainium2
NeuronCores.

Sharding: core c handles batch b = c//2 and a 4-head group g = c%2
(heads 4g..4g+3, i.e. output-feature slice [256g, 256g+256)).  Each core's
output is a disjoint slice of the full output, so no collectives are needed.

Device kernel layout (per core), v2 — flat software pipeline:
  - inputs are passed transposed+bf16 (xT = x.T : [din, S]) so the
    projection matmuls can contract din on the partition dim.
  - Q,K are produced transposed ("QT/KT" = [dout, S]); attention scores are
    computed transposed: ST[k, q] = sum_d KT[d,k] * QT[d,q].
  - softmax skips the max-subtraction (|s/8| <~ 6 for N(0,1)-ish inputs);
    exp runs on the scalar engine straight out of PSUM.
  - V is augmented with a ones-column so the PV matmul also accumulates the
    softmax denominator; normalization + transpose happen on the host.
  - the PE and ACT engines process attention columns at the same rate
    (0.833 ns/col), so the schedule is a single flat pipeline over all
    (head, q-half) pairs with one-group scores lookahead; projection tiles
    are placed as PE filler exactly where the ACT per-instruction overhead
    would otherwise stall the PE stream.
"""

import numpy as np
import ml_dtypes

from concourse import bacc, mybir
from concourse.tile import TileContext
from concourse.bass_utils import run_bass_kernel_spmd

BF16 = mybir.dt.bfloat16
F32 = mybir.dt.float32
AF = mybir.ActivationFunctionType
BFNP = ml_dtypes.bfloat16

B, S, D = 4, 2048, 512
H, HD = 8, 64
HPC = 4                   # heads per core
DSL = HPC * HD            # 256-wide output-feature slice per core
N_CORES = 8
SCALE = float(HD) ** 0.5  # 8.0
QH_W = 1024               # q processed in two halves of 1024


# timing instrumentation only: emit the compute body N times (identical
# output; wall-clock delta between variants isolates device compute time)
PASSES = 1


def std_groups(qh):
    """Standard score groups for a q-half: list of groups; each group is a
    list of (kt, a, b) column ranges (absolute q coords) of k-tile kt."""
    Q0 = QH_W * qh
    kmax = 8 if qh == 0 else 16

    def ent(kt):
        K0 = 128 * kt
        return (kt, max(Q0, K0), Q0 + QH_W)

    if qh == 0:
        # pairs pack to exactly 1024 cols (896+128, 768+256, 640+384):
        # one fewer exp instruction than the naive grouping
        idx = [(0,), (1, 7), (2, 6), (3, 5), (4,)]
    else:
        idx = [(k,) for k in range(9)] + [(9, 15), (10, 14), (11, 13), (12,)]
    return [[ent(kt) for kt in g] for g in idx]


def last_groups():
    """Final half keeps low-kt-only trailing groups so cols [0:512) can
    flush early (kt>11 touch only cols >= 512)."""
    idx = [(k,) for k in range(12)] + [(12, 13), (14, 15)]
    Q0 = QH_W
    return [[(kt, max(Q0, 128 * kt), Q0 + QH_W) for kt in g] for g in idx]


def h0q0_groups():
    """First half: split so the first scores only need Q[:, 0:512]."""
    return [
        [(0, 0, 512)],
        [(1, 128, 512), (2, 256, 512), (3, 384, 512)],
        [(0, 512, 1024)],
        [(1, 512, 1024), (2, 512, 1024)],
        [(3, 512, 1024), (4, 512, 1024)],
        [(5, 640, 1024), (6, 768, 1024), (7, 896, 1024)],
    ]


def build_nc():
    nc = bacc.Bacc("TRN2", target_bir_lowering=False)

    qT = nc.declare_dram_parameter("qT", [D, S], BF16, isOutput=False)
    kTd = nc.declare_dram_parameter("kTd", [D, S], BF16, isOutput=False)
    vT = nc.declare_dram_parameter("vT", [D, S], BF16, isOutput=False)
    wqT = nc.declare_dram_parameter("wqT", [D, DSL], BF16, isOutput=False)
    wkT = nc.declare_dram_parameter("wkT", [D, DSL], BF16, isOutput=False)
    wvT = nc.declare_dram_parameter("wvT", [D, DSL], BF16, isOutput=False)
    # packed small tensors: [0:2]=bq, [2:4]=bk, [4:260]=bvb, [260:324]=mask(bf16 bits)
    smallp = nc.declare_dram_parameter("smallp", [128, 324], F32, isOutput=False)
    # rows [65h, 65h+64) = unnormalized out^T for head h; row 65h+64 = denom
    out_t = nc.declare_dram_parameter(
        "out_t", [HPC * (HD + 1), S], F32, isOutput=True
    )

    with TileContext(nc) as tc:
        with tc.tile_pool(name="const", bufs=1) as cpool:
            qT_sb = cpool.tile([128, 4, S], BF16, tag="qT_sb")
            kT_sb = cpool.tile([128, 4, S], BF16, tag="kT_sb")
            vT_sb = cpool.tile([128, 4, S], BF16, tag="vT_sb")
            wq_sb = cpool.tile([128, 4, DSL], BF16, tag="wq_sb")
            wk_sb = cpool.tile([128, 4, DSL], BF16, tag="wk_sb")
            wv_sb = cpool.tile([128, 4, DSL], BF16, tag="wv_sb")
            small_sb = cpool.tile([128, 324], F32, tag="small_sb")
            bq_sb = small_sb[:, 0:2]
            bk_sb = small_sb[:, 2:4]
            bvb_sb = small_sb[:, 4:260]
            mask_sb = small_sb[:, 260:324].bitcast(BF16)
            # projected tensors: chunk dim = head pair (dout 128-chunk)
            QT_sb = cpool.tile([128, 2, S], BF16, tag="QT_sb")
            KT_sb = cpool.tile([128, 2, S], BF16, tag="KT_sb")
            # V with ones column: [k-part, head, k-tile, dv+1]
            vaug_sb = cpool.tile([128, HPC, 16, HD + 1], BF16, tag="vaug_sb")

            # only the ones-column needs init; cols 0..63 are written by proj_v
            nc.vector.memset(vaug_sb[:, :, :, HD : HD + 1], 1.0)
            # warmup scratch: the PE clock ramps with continuous busy time
            # (0.65->1.2->2.4 GHz over ~3us), so spin the PE on throwaway
            # matmuls while the first DMAs are in flight
            warm_sb = cpool.tile([128, 256], BF16, tag="warm_sb")
            nc.vector.memset(warm_sb[:], 1.0)

            def load_w_dc(w_sb, wsrc, dc0, dc1):
                if dc1 - dc0 == 1:
                    nc.sync.dma_start(
                        w_sb[:, dc0, :], wsrc[128 * dc0 : 128 * dc0 + 128, :]
                    )
                else:
                    nc.sync.dma_start(
                        w_sb[:, dc0:dc1, :],
                        wsrc[128 * dc0 : 128 * dc1, :].rearrange(
                            "(c p) m -> p c m", p=128
                        ),
                    )

            def load_x(dstt, srcd, s0, s1):
                nc.sync.dma_start(
                    dstt[:, :, s0:s1],
                    srcd[:, s0:s1].rearrange("(c p) s -> p c s", p=128),
                )

            # DMA order: prioritize the K->Q scores chain, V next, big tails
            # last.  Few large copies: every DMA pays a ~0.65us HWDGE slot,
            # so splitting loads only delays the chain's tail.
            load_w_dc(wk_sb, wkT, 0, 4)
            load_x(kT_sb, kTd, 0, 512)
            nc.sync.dma_start(small_sb[:], smallp[:])
            load_w_dc(wq_sb, wqT, 0, 4)
            load_x(qT_sb, qT, 0, 512)
            load_x(qT_sb, qT, 512, 1024)
            load_x(kT_sb, kTd, 512, 1024)
            load_w_dc(wv_sb, wvT, 0, 4)
            load_x(vT_sb, vT, 0, 512)
            load_x(vT_sb, vT, 512, 1024)
            load_x(qT_sb, qT, 1024, 2048)
            load_x(kT_sb, kTd, 1024, 2048)
            load_x(vT_sb, vT, 1024, 2048)

            # ---- projections + attention, flat pipeline ----
            # PSUM budget: ppool 2x1 + spool 2x2 + apool 1x2 = 8 banks
            with (
                tc.tile_pool(name="ppsum", bufs=2, space="PSUM") as ppool,
                tc.tile_pool(name="spsum", bufs=2, space="PSUM") as spool,
                tc.tile_pool(name="apsum", bufs=1, space="PSUM") as apool,
                tc.tile_pool(name="epool", bufs=12) as epool,
                tc.tile_pool(name="opool", bufs=6) as opool,
            ):

                def proj_v(st, h0, nh):
                    """V projection for seq window st, heads [h0, h0+nh)."""
                    c0, w = HD * h0, HD * nh
                    ps = ppool.tile([128, 512], F32, tag="pproj", name="psv")
                    for dc in range(4):
                        nc.tensor.matmul(
                            ps[:, 0:w],
                            vT_sb[:, dc, 128 * st : 128 * st + 128],
                            wv_sb[:, dc, c0 : c0 + w],
                            start=(dc == 0),
                            stop=(dc == 3),
                        )
                    nc.vector.tensor_add(
                        vaug_sb[:, h0 : h0 + nh, st, 0:HD],
                        ps[:, 0:w].rearrange("p (h d) -> p h d", h=nh),
                        bvb_sb[:, c0 : c0 + w].rearrange("p (h d) -> p h d", h=nh),
                    )

                QSRC = (wq_sb, bq_sb, qT_sb, QT_sb)
                KSRC = (wk_sb, bk_sb, kT_sb, KT_sb)

                def proj_qk_tile(mc, sc, src):
                    w_sb, b_sb, x_sb, dst = src
                    ps = ppool.tile([128, 512], F32, tag="pproj", name="psqk")
                    for dc in range(4):
                        nc.tensor.matmul(
                            ps[:],
                            w_sb[:, dc, 128 * mc : 128 * mc + 128],
                            x_sb[:, dc, 512 * sc : 512 * sc + 512],
                            start=(dc == 0),
                            stop=(dc == 3),
                        )
                    nc.vector.tensor_scalar_add(
                        dst[:, mc, 512 * sc : 512 * sc + 512],
                        ps[:],
                        b_sb[:, mc : mc + 1],
                    )

                def grp_offsets(grp):
                    # pack members tightly; a scores region must not
                    # cross a 512-element PSUM bank boundary
                    pos, offs = 0, []
                    for kt, a, b in grp:
                        W = b - a
                        if pos % 512 + min(W, 512) > 512:
                            pos = (pos + 511) // 512 * 512
                        offs.append(pos)
                        pos += W
                    return offs, pos

                def scores_grp(h, grp):
                    mc, prow = h // 2, 64 * (h % 2)
                    sl = spool.tile([128, QH_W], F32, tag="sl", name="sl")
                    offs = grp_offsets(grp)[0]
                    for (kt, a, b), base in zip(grp, offs):
                        K0, W = 128 * kt, b - a
                        for c0 in range(0, W, 512):
                            cw = min(512, W - c0)
                            nc.tensor.matmul(
                                sl[:, base + c0 : base + c0 + cw],
                                KT_sb[prow : prow + 64, mc, K0 : K0 + 128],
                                QT_sb[prow : prow + 64, mc, a + c0 : a + c0 + cw],
                                start=True,
                                stop=True,
                            )
                    return sl

                # ---- flat schedule over (head, q-half) pairs ----
                HALVES = [(0, 0), (1, 0), (0, 1), (1, 1),
                          (2, 0), (3, 0), (2, 1), (3, 1)]

                def qk(mc, sc, s):
                    return lambda: proj_qk_tile(mc, sc, s)

                def pv(st, h0, nh):
                    return lambda: proj_v(st, h0, nh)

                def make_scheds():
                    # per-half dict: group index -> ("slot", [thunks]) where
                    # slot M = after the scores lookahead, before this
                    # group's PV (PE filler while ACT runs exp); slot T =
                    # after this group's PV (for tiles whose input DMA lands
                    # later — keeps them off the PV critical path)
                    M, T = "M", "T"
                    return {
                        (0, 0): {
                            0: [(M, qk(0, 1, QSRC)), (T, qk(1, 0, QSRC))],
                            2: [(M, qk(0, 1, KSRC))],
                            3: [(M, qk(1, 1, QSRC))],
                            4: [(M, pv(0, 0, 2)), (M, pv(1, 0, 2)),
                                (M, pv(2, 0, 2)), (M, pv(3, 0, 2))],
                            5: [(M, pv(4, 0, 2)), (M, pv(5, 0, 2)),
                                (M, qk(1, 1, KSRC))],
                        },
                        (1, 0): {
                            0: [(M, pv(6, 0, 2)), (M, pv(7, 0, 2))],
                            2: [(M, qk(0, 2, QSRC))],
                            3: [(M, qk(0, 3, QSRC))],
                        },
                        (0, 1): {
                            0: [(M, qk(1, 2, QSRC))],
                            2: [(M, qk(1, 3, QSRC))],
                            5: [(M, qk(0, 2, KSRC)), (M, pv(8, 0, 2))],
                            6: [(M, qk(0, 3, KSRC)), (M, pv(9, 0, 2))],
                            7: [(M, pv(10, 0, 2))],
                            8: [(M, pv(11, 0, 2))],
                            9: [(M, pv(12, 0, 2))],
                            10: [(M, pv(13, 0, 2))],
                            11: [(M, pv(14, 0, 2))],
                            12: [(M, pv(15, 0, 2))],
                        },
                        (1, 1): {
                            0: [(M, qk(1, 2, KSRC))],
                            4: [(M, qk(1, 3, KSRC))],
                        },
                        (2, 0): {
                            0: [(M, pv(0, 2, 2))],
                            1: [(M, pv(1, 2, 2)), (M, pv(2, 2, 2))],
                            2: [(M, pv(3, 2, 2)), (M, pv(4, 2, 2))],
                            3: [(M, pv(5, 2, 2)), (M, pv(6, 2, 2))],
                            4: [(M, pv(7, 2, 2))],
                        },
                        (3, 0): {
                            0: [(M, pv(8, 2, 1))],
                            1: [(M, pv(9, 2, 1)), (M, pv(10, 2, 1))],
                            2: [(M, pv(11, 2, 1)), (M, pv(8, 3, 1))],
                            3: [(M, pv(9, 3, 1)), (M, pv(10, 3, 1))],
                            4: [(M, pv(11, 3, 1))],
                        },
                        (2, 1): {
                            8: [(M, pv(12, 2, 1))],
                            9: [(M, pv(13, 2, 1))],
                            10: [(M, pv(14, 2, 1))],
                            11: [(M, pv(15, 2, 1))],
                        },
                        (3, 1): {
                            8: [(M, pv(12, 3, 1))],
                            9: [(M, pv(13, 3, 1))],
                            10: [(M, pv(14, 3, 1))],
                            11: [(M, pv(15, 3, 1))],
                        },
                    }

                def flush(h, qh, acc, c0, c1):
                    # acc is (accA, accB): separate 1-bank tiles for q-cols
                    # [0:512) and [512:1024) so flush copies never couple
                    # with PV writes of the other half through tile deps
                    ot = opool.tile([HD + 1, 512], F32, tag="ot", name="ot")
                    t = acc[c0 // 512]
                    nc.vector.tensor_copy(ot[:, 0 : c1 - c0],
                                          t[:, c0 % 512 : c0 % 512 + c1 - c0])
                    r0 = (HD + 1) * h
                    Q0 = QH_W * qh
                    nc.sync.dma_start(
                        out_t[r0 : r0 + HD + 1, Q0 + c0 : Q0 + c1],
                        ot[:, 0 : c1 - c0],
                    )

                def emit_pv(rec):
                    # PV matmuls for a group, several pipeline stages behind
                    # the exp: everything here is ready, so the PE never
                    # blocks
                    h, qh, grp, goffs, et, acc, post = rec
                    Q0 = QH_W * qh
                    kmax = 8 if qh == 0 else 16
                    for (kt, a, b), base in zip(grp, goffs):
                        off = a - Q0
                        b0 = off
                        while b0 < off + (b - a):
                            b1 = min(off + (b - a), (b0 // 512 + 1) * 512)
                            nc.tensor.matmul(
                                acc[b0 // 512][:, b0 % 512 : b0 % 512 + b1 - b0],
                                vaug_sb[:, h, kt, :],
                                et[:, base + b0 - off : base + b1 - off],
                                start=(kt == 0),
                                stop=(kt == kmax - 1),
                                skip_group_check=True,
                            )
                            b0 = b1
                    for fl in post:
                        fl()

                for _pass in range(PASSES):
                    scheds = make_scheds()
                    groups_of = {
                        hq: (h0q0_groups() if hq == (0, 0)
                             else last_groups() if hq == HALVES[-1]
                             else std_groups(hq[1]))
                        for hq in HALVES
                    }
                    # prologue: PE warmup spin (clock ramp) while DMAs land,
                    # then the projections feeding the very first scores;
                    # K(mc1) last so its bias-add doesn't gate the Q tile
                    # through the ppool rotation
                    wps = ppool.tile([128, 512], F32, tag="pproj", name="warm")
                    for _w in range(14):
                        nc.tensor.matmul(
                            wps[0:1, 0:256], warm_sb[:, 0:1], warm_sb[:],
                            start=True, stop=True,
                        )
                    proj_qk_tile(0, 0, KSRC)
                    proj_qk_tile(0, 0, QSRC)
                    proj_qk_tile(1, 0, KSRC)

                    pending_sl = scores_grp(HALVES[0][0], groups_of[HALVES[0]][0])
                    pvq = []  # deferred PV records (deeper lag at startup
                    #           so early PVs don't block the PE stream while
                    #           the V DMAs are still in flight)
                    for hi, (h, qh) in enumerate(HALVES):
                        groups = groups_of[(h, qh)]
                        sched = scheds[(h, qh)]
                        acc = (apool.tile([HD + 1, 512], F32, tag="accA",
                                          name="accA"),
                               apool.tile([HD + 1, 512], F32, tag="accB",
                                          name="accB"))
                        last_half = hi + 1 >= len(HALVES)
                        for gi, grp in enumerate(groups):
                            sl = pending_sl
                            goffs, We = grp_offsets(grp)
                            et = epool.tile([128, QH_W], BF16, tag="et",
                                            name="et")
                            nc.scalar.activation(
                                et[:, 0:We], sl[:, 0:We], AF.Exp,
                                scale=1.0 / SCALE,
                            )
                            for (kt, a, b), base in zip(grp, goffs):
                                if a == 128 * kt:  # diagonal: causal mask
                                    nc.vector.tensor_mul(
                                        et[:, base : base + 128],
                                        et[:, base : base + 128],
                                        mask_sb[:],
                                    )
                            # one-group scores lookahead (crosses halves)
                            if gi + 1 < len(groups):
                                pending_sl = scores_grp(h, groups[gi + 1])
                            elif not last_half:
                                nh, nqh = HALVES[hi + 1]
                                pending_sl = scores_grp(
                                    nh, groups_of[(nh, nqh)][0]
                                )
                            else:
                                pending_sl = None
                            for slot, work in sched.get(gi, ()):
                                if slot == "M":
                                    work()
                            if hi == 0 and gi < 4:
                                lag = 99
                            elif last_half and gi >= 8:
                                lag = max(1, 13 - gi)
                            else:
                                lag = 8
                            while len(pvq) > lag:
                                emit_pv(pvq.pop(0))
                            for slot, work in sched.get(gi, ()):
                                if slot == "T":
                                    work()
                            # flushes ride on the PV record (run after it);
                            # the final half streams out in 4 chunks, each as
                            # soon as its last k-tile has accumulated
                            post = []

                            def fl(c0, c1, h_=h, q_=qh, a_=acc):
                                return lambda: flush(h_, q_, a_, c0, c1)

                            # cols [0:512) finish early (only low k-tiles
                            # touch them): flush A as soon as its last
                            # contributor's PV lands, B at the half's end
                            if hi == 0:
                                a_gi = 1
                            elif qh == 0:
                                a_gi = 3
                            else:
                                a_gi = 11
                            if gi == a_gi:
                                post.append(fl(0, 512))
                            if gi + 1 == len(groups):
                                post.append(fl(512, QH_W))
                            pvq.append((h, qh, grp, goffs, et, acc, post))
                    # drain the final PVs after the last exp
                    for rec in pvq:
                        emit_pv(rec)

    nc.finalize()
    return nc


_NC_CACHE = {}


def _get_nc():
    if "nc" not in _NC_CACHE:
        _NC_CACHE["nc"] = build_nc()
    return _NC_CACHE["nc"]


def make_in_maps(query, key, value, Wq, bq, Wk, bk, Wv, bv):
    query, key, value = (np.asarray(x, np.float32) for x in (query, key, value))
    Wq, Wk, Wv = (np.asarray(x, np.float32) for x in (Wq, Wk, Wv))
    bq, bk, bv = (np.asarray(x, np.float32) for x in (bq, bk, bv))
    mask = np.triu(np.ones((128, 128), np.float32)).astype(BFNP)

    def pack_small(bqs, bks, bvs, m):
        out = np.empty((128, 324), np.float32)
        out[:, 0:2] = bqs.reshape(2, 128).T
        out[:, 2:4] = bks.reshape(2, 128).T
        out[:, 4:260] = np.tile(bvs[None, :], (128, 1))
        out[:, 260:324] = np.ascontiguousarray(m).view(np.float32)
        return out

    in_maps = []
    for c in range(N_CORES):
        b, g = c // 2, c % 2
        sl = slice(DSL * g, DSL * g + DSL)
        in_maps.append(
            {
                "qT": np.ascontiguousarray(query[b].astype(BFNP).T),
                "kTd": np.ascontiguousarray(key[b].astype(BFNP).T),
                "vT": np.ascontiguousarray(value[b].astype(BFNP).T),
                "wqT": np.ascontiguousarray(Wq[sl].astype(BFNP).T),
                "wkT": np.ascontiguousarray(Wk[sl].astype(BFNP).T),
                "wvT": np.ascontiguousarray(Wv[sl].astype(BFNP).T),
                "smallp": pack_small(bq[sl], bk[sl], bv[sl], mask),
            }
        )
    return in_maps


def assemble_output(results):
    out = np.empty((B, S, D), np.float32)
    for c in range(N_CORES):
        b, g = c // 2, c % 2
        ot = results[c]["out_t"]  # [260, 2048]
        for hl in range(HPC):
            blk = ot[(HD + 1) * hl : (HD + 1) * hl + HD]  # [64, S]
            den = ot[(HD + 1) * hl + HD]  # [S]
            h = HPC * g + hl
            out[b, :, HD * h : HD * h + HD] = (blk / den).T
    return out


def run(trace=False, **inputs):
    nc = _get_nc()
    in_maps = make_in_maps(**inputs)
    res = run_bass_kernel_spmd(nc, in_maps, list(range(N_CORES)), trace=trace)
    return assemble_output(res.results), res


def kernel(**inputs) -> np.ndarray:
    out, _ = run(trace=False, **inputs)
    return out


# revision 9
# speedup vs baseline: 1.1849x; 1.0083x over previous
"""Multi-head causal self-attention (B=4, S=2048, D=512, H=8) on 8 Trainium2
NeuronCores.

Sharding: core c handles batch b = c//2 and a 4-head group g = c%2
(heads 4g..4g+3, i.e. output-feature slice [256g, 256g+256)).  Each core's
output is a disjoint slice of the full output, so no collectives are needed.

Device kernel layout (per core), v2 — flat software pipeline:
  - inputs are passed transposed+bf16 (xT = x.T : [din, S]) so the
    projection matmuls can contract din on the partition dim.
  - Q,K are produced transposed ("QT/KT" = [dout, S]); attention scores are
    computed transposed: ST[k, q] = sum_d KT[d,k] * QT[d,q].
  - softmax skips the max-subtraction (|s/8| <~ 6 for N(0,1)-ish inputs);
    exp runs on the scalar engine straight out of PSUM.
  - V is augmented with a ones-column so the PV matmul also accumulates the
    softmax denominator; normalization + transpose happen on the host.
  - the PE and ACT engines process attention columns at the same rate
    (0.833 ns/col), so the schedule is a single flat pipeline over all
    (head, q-half) pairs with one-group scores lookahead; projection tiles
    are placed as PE filler exactly where the ACT per-instruction overhead
    would otherwise stall the PE stream.
"""

import numpy as np
import ml_dtypes

from concourse import bacc, mybir
from concourse.tile import TileContext
from concourse.bass_utils import run_bass_kernel_spmd

BF16 = mybir.dt.bfloat16
F32 = mybir.dt.float32
AF = mybir.ActivationFunctionType
BFNP = ml_dtypes.bfloat16

B, S, D = 4, 2048, 512
H, HD = 8, 64
HPC = 4                   # heads per core
DSL = HPC * HD            # 256-wide output-feature slice per core
N_CORES = 8
SCALE = float(HD) ** 0.5  # 8.0
QH_W = 1024               # q processed in two halves of 1024


# timing instrumentation only: emit the compute body N times (identical
# output; wall-clock delta between variants isolates device compute time)
PASSES = 1


def std_groups(qh):
    """Standard score groups for a q-half: list of groups; each group is a
    list of (kt, a, b) column ranges (absolute q coords) of k-tile kt."""
    Q0 = QH_W * qh
    kmax = 8 if qh == 0 else 16

    def ent(kt):
        K0 = 128 * kt
        return (kt, max(Q0, K0), Q0 + QH_W)

    if qh == 0:
        # pairs pack to exactly 1024 cols (896+128, 768+256, 640+384):
        # one fewer exp instruction than the naive grouping
        idx = [(0,), (1, 7), (2, 6), (3, 5), (4,)]
    else:
        idx = [(k,) for k in range(9)] + [(9, 15), (10, 14), (11, 13), (12,)]
    return [[ent(kt) for kt in g] for g in idx]


def last_groups():
    """Final half keeps low-kt-only trailing groups so cols [0:512) can
    flush early (kt>11 touch only cols >= 512); (14) and (15) stay separate
    so cols [512:896) flush while the tiny kt15 tail finishes."""
    idx = [(k,) for k in range(12)] + [(12, 13), (14, 15)]
    Q0 = QH_W
    return [[(kt, max(Q0, 128 * kt), Q0 + QH_W) for kt in g] for g in idx]


def h0q0_groups():
    """First half: split so the first scores only need Q[:, 0:512]."""
    return [
        [(0, 0, 512)],
        [(1, 128, 512), (2, 256, 512), (3, 384, 512)],
        [(0, 512, 1024)],
        [(1, 512, 1024), (2, 512, 1024)],
        [(3, 512, 1024), (4, 512, 1024)],
        [(5, 640, 1024), (6, 768, 1024), (7, 896, 1024)],
    ]


def build_nc():
    nc = bacc.Bacc("TRN2", target_bir_lowering=False)

    qT = nc.declare_dram_parameter("qT", [D, S], BF16, isOutput=False)
    kTd = nc.declare_dram_parameter("kTd", [D, S], BF16, isOutput=False)
    vT = nc.declare_dram_parameter("vT", [D, S], BF16, isOutput=False)
    wqT = nc.declare_dram_parameter("wqT", [D, DSL], BF16, isOutput=False)
    wkT = nc.declare_dram_parameter("wkT", [D, DSL], BF16, isOutput=False)
    wvT = nc.declare_dram_parameter("wvT", [D, DSL], BF16, isOutput=False)
    # packed small tensors: [0:2]=bq, [2:4]=bk, [4:132]=bvb (bf16 bits),
    # [132:196]=mask (bf16 bits)
    smallp = nc.declare_dram_parameter("smallp", [128, 196], F32, isOutput=False)
    # rows [65h, 65h+64) = unnormalized out^T for head h; row 65h+64 = denom
    out_t = nc.declare_dram_parameter(
        "out_t", [HPC * (HD + 1), S], F32, isOutput=True
    )

    with TileContext(nc) as tc:
        with tc.tile_pool(name="const", bufs=1) as cpool:
            qT_sb = cpool.tile([128, 4, S], BF16, tag="qT_sb")
            kT_sb = cpool.tile([128, 4, S], BF16, tag="kT_sb")
            vT_sb = cpool.tile([128, 4, S], BF16, tag="vT_sb")
            wq_sb = cpool.tile([128, 4, DSL], BF16, tag="wq_sb")
            wk_sb = cpool.tile([128, 4, DSL], BF16, tag="wk_sb")
            wv_sb = cpool.tile([128, 4, DSL], BF16, tag="wv_sb")
            small_sb = cpool.tile([128, 196], F32, tag="small_sb")
            bq_sb = small_sb[:, 0:2]
            bk_sb = small_sb[:, 2:4]
            bvb_sb = small_sb[:, 4:132].bitcast(BF16)
            mask_sb = small_sb[:, 132:196].bitcast(BF16)
            # projected tensors: chunk dim = head pair (dout 128-chunk)
            QT_sb = cpool.tile([128, 2, S], BF16, tag="QT_sb")
            KT_sb = cpool.tile([128, 2, S], BF16, tag="KT_sb")
            # V with ones column: [k-part, head, k-tile, dv+1]
            vaug_sb = cpool.tile([128, HPC, 16, HD + 1], BF16, tag="vaug_sb")

            # only the ones-column needs init; cols 0..63 are written by proj_v
            nc.vector.memset(vaug_sb[:, :, :, HD : HD + 1], 1.0)
            # warmup scratch: the PE clock ramps with continuous busy time
            # (0.65->1.2->2.4 GHz over ~3us), so spin the PE on throwaway
            # matmuls while the first DMAs are in flight
            warm_sb = cpool.tile([128, 256], BF16, tag="warm_sb")
            nc.vector.memset(warm_sb[:], 1.0)

            def load_w_dc(w_sb, wsrc, dc0, dc1):
                if dc1 - dc0 == 1:
                    nc.sync.dma_start(
                        w_sb[:, dc0, :], wsrc[128 * dc0 : 128 * dc0 + 128, :]
                    )
                else:
                    nc.sync.dma_start(
                        w_sb[:, dc0:dc1, :],
                        wsrc[128 * dc0 : 128 * dc1, :].rearrange(
                            "(c p) m -> p c m", p=128
                        ),
                    )

            def load_x(dstt, srcd, s0, s1):
                nc.sync.dma_start(
                    dstt[:, :, s0:s1],
                    srcd[:, s0:s1].rearrange("(c p) s -> p c s", p=128),
                )

            # DMA order: prioritize the K->Q scores chain, V next, big tails
            # last.  Few large copies: every DMA pays a ~0.65us HWDGE slot,
            # so splitting loads only delays the chain's tail.
            load_w_dc(wk_sb, wkT, 0, 4)
            load_x(kT_sb, kTd, 0, 512)
            nc.sync.dma_start(small_sb[:], smallp[:])
            load_w_dc(wq_sb, wqT, 0, 4)
            load_x(qT_sb, qT, 0, 512)
            load_x(qT_sb, qT, 512, 1024)
            load_x(kT_sb, kTd, 512, 1024)
            load_w_dc(wv_sb, wvT, 0, 4)
            load_x(vT_sb, vT, 0, 512)
            load_x(vT_sb, vT, 512, 1024)
            load_x(qT_sb, qT, 1024, 2048)
            load_x(kT_sb, kTd, 1024, 2048)
            load_x(vT_sb, vT, 1024, 2048)

            # ---- projections + attention, flat pipeline ----
            # PSUM budget: ppool 2x1 + spool 2x2 + apool 1x2 = 8 banks
            with (
                tc.tile_pool(name="ppsum", bufs=2, space="PSUM") as ppool,
                tc.tile_pool(name="spsum", bufs=2, space="PSUM") as spool,
                tc.tile_pool(name="apsum", bufs=1, space="PSUM") as apool,
                tc.tile_pool(name="epool", bufs=12) as epool,
                tc.tile_pool(name="opool", bufs=6) as opool,
            ):

                def proj_v(st, h0, nh):
                    """V projection for seq window st, heads [h0, h0+nh)."""
                    c0, w = HD * h0, HD * nh
                    ps = ppool.tile([128, 512], F32, tag="pproj", name="psv")
                    for dc in range(4):
                        nc.tensor.matmul(
                            ps[:, 0:w],
                            vT_sb[:, dc, 128 * st : 128 * st + 128],
                            wv_sb[:, dc, c0 : c0 + w],
                            start=(dc == 0),
                            stop=(dc == 3),
                        )
                    nc.vector.tensor_add(
                        vaug_sb[:, h0 : h0 + nh, st, 0:HD],
                        ps[:, 0:w].rearrange("p (h d) -> p h d", h=nh),
                        bvb_sb[:, c0 : c0 + w].rearrange("p (h d) -> p h d", h=nh),
                    )

                QSRC = (wq_sb, bq_sb, qT_sb, QT_sb)
                KSRC = (wk_sb, bk_sb, kT_sb, KT_sb)

                def proj_qk_tile(mc, sc, src):
                    w_sb, b_sb, x_sb, dst = src
                    ps = ppool.tile([128, 512], F32, tag="pproj", name="psqk")
                    for dc in range(4):
                        nc.tensor.matmul(
                            ps[:],
                            w_sb[:, dc, 128 * mc : 128 * mc + 128],
                            x_sb[:, dc, 512 * sc : 512 * sc + 512],
                            start=(dc == 0),
                            stop=(dc == 3),
                        )
                    nc.vector.tensor_scalar_add(
                        dst[:, mc, 512 * sc : 512 * sc + 512],
                        ps[:],
                        b_sb[:, mc : mc + 1],
                    )

                def grp_offsets(grp):
                    # pack members tightly; a scores region must not
                    # cross a 512-element PSUM bank boundary
                    pos, offs = 0, []
                    for kt, a, b in grp:
                        W = b - a
                        if pos % 512 + min(W, 512) > 512:
                            pos = (pos + 511) // 512 * 512
                        offs.append(pos)
                        pos += W
                    return offs, pos

                def scores_grp(h, grp):
                    mc, prow = h // 2, 64 * (h % 2)
                    sl = spool.tile([128, QH_W], F32, tag="sl", name="sl")
                    offs = grp_offsets(grp)[0]
                    for (kt, a, b), base in zip(grp, offs):
                        K0, W = 128 * kt, b - a
                        for c0 in range(0, W, 512):
                            cw = min(512, W - c0)
                            nc.tensor.matmul(
                                sl[:, base + c0 : base + c0 + cw],
                                KT_sb[prow : prow + 64, mc, K0 : K0 + 128],
                                QT_sb[prow : prow + 64, mc, a + c0 : a + c0 + cw],
                                start=True,
                                stop=True,
                            )
                    return sl

                # ---- flat schedule over (head, q-half) pairs ----
                HALVES = [(0, 0), (1, 0), (0, 1), (1, 1),
                          (2, 0), (3, 0), (2, 1), (3, 1)]

                def qk(mc, sc, s):
                    return lambda: proj_qk_tile(mc, sc, s)

                def pv(st, h0, nh):
                    return lambda: proj_v(st, h0, nh)

                def make_scheds():
                    # per-half dict: group index -> ("slot", [thunks]) where
                    # slot M = after the scores lookahead, before this
                    # group's PV (PE filler while ACT runs exp); slot T =
                    # after this group's PV (for tiles whose input DMA lands
                    # later — keeps them off the PV critical path)
                    M, T = "M", "T"
                    return {
                        (0, 0): {
                            0: [(M, qk(0, 1, QSRC)), (T, qk(1, 0, QSRC))],
                            2: [(M, qk(0, 1, KSRC))],
                            3: [(M, qk(1, 1, QSRC))],
                            4: [(M, pv(0, 0, 2)), (M, pv(1, 0, 2)),
                                (M, pv(2, 0, 2)), (M, pv(3, 0, 2))],
                            5: [(M, pv(4, 0, 2)), (M, pv(5, 0, 2)),
                                (M, qk(1, 1, KSRC))],
                        },
                        (1, 0): {
                            0: [(M, pv(6, 0, 2)), (M, pv(7, 0, 2))],
                            2: [(M, qk(0, 2, QSRC))],
                            3: [(M, qk(0, 3, QSRC))],
                        },
                        (0, 1): {
                            0: [(M, qk(1, 2, QSRC))],
                            2: [(M, qk(1, 3, QSRC))],
                            5: [(M, qk(0, 2, KSRC)), (M, pv(8, 0, 2))],
                            6: [(M, qk(0, 3, KSRC)), (M, pv(9, 0, 2))],
                            7: [(M, pv(10, 0, 2))],
                            8: [(M, pv(11, 0, 2))],
                            9: [(M, pv(12, 0, 2))],
                            10: [(M, pv(13, 0, 2))],
                            11: [(M, pv(14, 0, 2))],
                            12: [(M, pv(15, 0, 2))],
                        },
                        (1, 1): {
                            0: [(M, qk(1, 2, KSRC))],
                            4: [(M, qk(1, 3, KSRC))],
                        },
                        (2, 0): {
                            0: [(M, pv(0, 2, 2))],
                            1: [(M, pv(1, 2, 2)), (M, pv(2, 2, 2))],
                            2: [(M, pv(3, 2, 2)), (M, pv(4, 2, 2))],
                            3: [(M, pv(5, 2, 2)), (M, pv(6, 2, 2))],
                            4: [(M, pv(7, 2, 2))],
                        },
                        (3, 0): {
                            0: [(M, pv(8, 2, 1))],
                            1: [(M, pv(9, 2, 1)), (M, pv(10, 2, 1))],
                            2: [(M, pv(11, 2, 1)), (M, pv(8, 3, 1))],
                            3: [(M, pv(9, 3, 1)), (M, pv(10, 3, 1))],
                            4: [(M, pv(11, 3, 1))],
                        },
                        (2, 1): {
                            8: [(M, pv(12, 2, 1))],
                            9: [(M, pv(13, 2, 1))],
                            10: [(M, pv(14, 2, 1))],
                            11: [(M, pv(15, 2, 1))],
                        },
                        (3, 1): {
                            8: [(M, pv(12, 3, 1))],
                            9: [(M, pv(13, 3, 1))],
                            10: [(M, pv(14, 3, 1))],
                            11: [(M, pv(15, 3, 1))],
                        },
                    }

                def flush(h, qh, acc, c0, c1):
                    # acc is (accA, accB): separate 1-bank tiles for q-cols
                    # [0:512) and [512:1024) so flush copies never couple
                    # with PV writes of the other half through tile deps
                    ot = opool.tile([HD + 1, 512], F32, tag="ot", name="ot")
                    t = acc[c0 // 512]
                    nc.vector.tensor_copy(ot[:, 0 : c1 - c0],
                                          t[:, c0 % 512 : c0 % 512 + c1 - c0])
                    r0 = (HD + 1) * h
                    Q0 = QH_W * qh
                    nc.sync.dma_start(
                        out_t[r0 : r0 + HD + 1, Q0 + c0 : Q0 + c1],
                        ot[:, 0 : c1 - c0],
                    )

                def emit_pv(rec):
                    # PV matmuls for a group, several pipeline stages behind
                    # the exp: everything here is ready, so the PE never
                    # blocks
                    h, qh, grp, goffs, et, acc, post = rec
                    Q0 = QH_W * qh
                    kmax = 8 if qh == 0 else 16
                    for (kt, a, b), base in zip(grp, goffs):
                        off = a - Q0
                        b0 = off
                        while b0 < off + (b - a):
                            b1 = min(off + (b - a), (b0 // 512 + 1) * 512)
                            nc.tensor.matmul(
                                acc[b0 // 512][:, b0 % 512 : b0 % 512 + b1 - b0],
                                vaug_sb[:, h, kt, :],
                                et[:, base + b0 - off : base + b1 - off],
                                start=(kt == 0),
                                stop=(kt == kmax - 1),
                                skip_group_check=True,
                            )
                            b0 = b1
                    for fl in post:
                        fl()

                for _pass in range(PASSES):
                    scheds = make_scheds()
                    groups_of = {
                        hq: (h0q0_groups() if hq == (0, 0)
                             else last_groups() if hq == HALVES[-1]
                             else std_groups(hq[1]))
                        for hq in HALVES
                    }
                    # prologue: PE warmup spin (clock ramp) while DMAs land,
                    # then the projections feeding the very first scores;
                    # K(mc1) last so its bias-add doesn't gate the Q tile
                    # through the ppool rotation
                    wps = ppool.tile([128, 512], F32, tag="pproj", name="warm")
                    for _w in range(14):
                        nc.tensor.matmul(
                            wps[0:1, 0:256], warm_sb[:, 0:1], warm_sb[:],
                            start=True, stop=True,
                        )
                    proj_qk_tile(0, 0, KSRC)
                    proj_qk_tile(0, 0, QSRC)
                    proj_qk_tile(1, 0, KSRC)

                    pending_sl = scores_grp(HALVES[0][0], groups_of[HALVES[0]][0])
                    pvq = []  # deferred PV records (deeper lag at startup
                    #           so early PVs don't block the PE stream while
                    #           the V DMAs are still in flight)
                    for hi, (h, qh) in enumerate(HALVES):
                        groups = groups_of[(h, qh)]
                        sched = scheds[(h, qh)]
                        acc = (apool.tile([HD + 1, 512], F32, tag="accA",
                                          name="accA"),
                               apool.tile([HD + 1, 512], F32, tag="accB",
                                          name="accB"))
                        last_half = hi + 1 >= len(HALVES)
                        for gi, grp in enumerate(groups):
                            sl = pending_sl
                            goffs, We = grp_offsets(grp)
                            et = epool.tile([128, QH_W], BF16, tag="et",
                                            name="et")
                            nc.scalar.activation(
                                et[:, 0:We], sl[:, 0:We], AF.Exp,
                                scale=1.0 / SCALE,
                            )
                            for (kt, a, b), base in zip(grp, goffs):
                                if a == 128 * kt:  # diagonal: causal mask
                                    nc.vector.tensor_mul(
                                        et[:, base : base + 128],
                                        et[:, base : base + 128],
                                        mask_sb[:],
                                    )
                            # one-group scores lookahead (crosses halves)
                            if gi + 1 < len(groups):
                                pending_sl = scores_grp(h, groups[gi + 1])
                            elif not last_half:
                                nh, nqh = HALVES[hi + 1]
                                pending_sl = scores_grp(
                                    nh, groups_of[(nh, nqh)][0]
                                )
                            else:
                                pending_sl = None
                            for slot, work in sched.get(gi, ()):
                                if slot == "M":
                                    work()
                            if hi == 0 and gi < 4:
                                lag = 99
                            elif last_half and gi >= 8:
                                lag = max(1, 13 - gi)
                            else:
                                lag = 9
                            while len(pvq) > lag:
                                emit_pv(pvq.pop(0))
                            for slot, work in sched.get(gi, ()):
                                if slot == "T":
                                    work()
                            # flushes ride on the PV record (run after it);
                            # the final half streams out in 4 chunks, each as
                            # soon as its last k-tile has accumulated
                            post = []

                            def fl(c0, c1, h_=h, q_=qh, a_=acc):
                                return lambda: flush(h_, q_, a_, c0, c1)

                            # cols [0:512) finish early (only low k-tiles
                            # touch them): flush A as soon as its last
                            # contributor's PV lands, B at the half's end
                            if hi == 0:
                                a_gi = 1
                            elif qh == 0:
                                a_gi = 3
                            else:
                                a_gi = 11
                            if gi == a_gi:
                                post.append(fl(0, 512))
                            if gi + 1 == len(groups):
                                post.append(fl(512, QH_W))
                            pvq.append((h, qh, grp, goffs, et, acc, post))
                    # drain the final PVs after the last exp
                    for rec in pvq:
                        emit_pv(rec)

    nc.finalize()
    return nc


_NC_CACHE = {}


def _get_nc():
    if "nc" not in _NC_CACHE:
        _NC_CACHE["nc"] = build_nc()
    return _NC_CACHE["nc"]


def make_in_maps(query, key, value, Wq, bq, Wk, bk, Wv, bv):
    query, key, value = (np.asarray(x, np.float32) for x in (query, key, value))
    Wq, Wk, Wv = (np.asarray(x, np.float32) for x in (Wq, Wk, Wv))
    bq, bk, bv = (np.asarray(x, np.float32) for x in (bq, bk, bv))
    mask = np.triu(np.ones((128, 128), np.float32)).astype(BFNP)

    def pack_small(bqs, bks, bvs, m):
        out = np.empty((128, 196), np.float32)
        out[:, 0:2] = bqs.reshape(2, 128).T
        out[:, 2:4] = bks.reshape(2, 128).T
        bvt = np.tile(bvs[None, :].astype(BFNP), (128, 1))
        out[:, 4:132] = np.ascontiguousarray(bvt).view(np.float32)
        out[:, 132:196] = np.ascontiguousarray(m).view(np.float32)
        return out

    in_maps = []
    for c in range(N_CORES):
        b, g = c // 2, c % 2
        sl = slice(DSL * g, DSL * g + DSL)
        in_maps.append(
            {
                "qT": np.ascontiguousarray(query[b].astype(BFNP).T),
                "kTd": np.ascontiguousarray(key[b].astype(BFNP).T),
                "vT": np.ascontiguousarray(value[b].astype(BFNP).T),
                "wqT": np.ascontiguousarray(Wq[sl].astype(BFNP).T),
                "wkT": np.ascontiguousarray(Wk[sl].astype(BFNP).T),
                "wvT": np.ascontiguousarray(Wv[sl].astype(BFNP).T),
                "smallp": pack_small(bq[sl], bk[sl], bv[sl], mask),
            }
        )
    return in_maps


def assemble_output(results):
    out = np.empty((B, S, D), np.float32)
    for c in range(N_CORES):
        b, g = c // 2, c % 2
        ot = results[c]["out_t"]  # [260, 2048]
        for hl in range(HPC):
            blk = ot[(HD + 1) * hl : (HD + 1) * hl + HD]  # [64, S]
            den = ot[(HD + 1) * hl + HD]  # [S]
            h = HPC * g + hl
            out[b, :, HD * h : HD * h + HD] = (blk / den).T
    return out


def run(trace=False, **inputs):
    nc = _get_nc()
    in_maps = make_in_maps(**inputs)
    res = run_bass_kernel_spmd(nc, in_maps, list(range(N_CORES)), trace=trace)
    return assemble_output(res.results), res


def kernel(**inputs) -> np.ndarray:
    out, _ = run(trace=False, **inputs)
    return out


# revision 10
# speedup vs baseline: 1.1852x; 1.0002x over previous
"""Multi-head causal self-attention (B=4, S=2048, D=512, H=8) on 8 Trainium2
NeuronCores.

Sharding: core c handles batch b = c//2 and a 4-head group g = c%2
(heads 4g..4g+3, i.e. output-feature slice [256g, 256g+256)).  Each core's
output is a disjoint slice of the full output, so no collectives are needed.

Device kernel layout (per core), v2 — flat software pipeline:
  - inputs are passed transposed+bf16 (xT = x.T : [din, S]) so the
    projection matmuls can contract din on the partition dim.
  - Q,K are produced transposed ("QT/KT" = [dout, S]); attention scores are
    computed transposed: ST[k, q] = sum_d KT[d,k] * QT[d,q].
  - softmax skips the max-subtraction (|s/8| <~ 6 for N(0,1)-ish inputs);
    exp runs on the scalar engine straight out of PSUM.
  - V is augmented with a ones-column so the PV matmul also accumulates the
    softmax denominator; normalization + transpose happen on the host.
  - the PE and ACT engines process attention columns at the same rate
    (0.833 ns/col), so the schedule is a single flat pipeline over all
    (head, q-half) pairs with one-group scores lookahead; projection tiles
    are placed as PE filler exactly where the ACT per-instruction overhead
    would otherwise stall the PE stream.
"""

import numpy as np
import ml_dtypes

from concourse import bacc, mybir
from concourse.tile import TileContext
from concourse.bass_utils import run_bass_kernel_spmd

BF16 = mybir.dt.bfloat16
F32 = mybir.dt.float32
AF = mybir.ActivationFunctionType
BFNP = ml_dtypes.bfloat16

B, S, D = 4, 2048, 512
H, HD = 8, 64
HPC = 4                   # heads per core
DSL = HPC * HD            # 256-wide output-feature slice per core
N_CORES = 8
SCALE = float(HD) ** 0.5  # 8.0
QH_W = 1024               # q processed in two halves of 1024


# timing instrumentation only: emit the compute body N times (identical
# output; wall-clock delta between variants isolates device compute time)
PASSES = 1


def std_groups(qh):
    """Standard score groups for a q-half: list of groups; each group is a
    list of (kt, a, b) column ranges (absolute q coords) of k-tile kt."""
    Q0 = QH_W * qh
    kmax = 8 if qh == 0 else 16

    def ent(kt):
        K0 = 128 * kt
        return (kt, max(Q0, K0), Q0 + QH_W)

    if qh == 0:
        # pairs pack to exactly 1024 cols (896+128, 768+256, 640+384):
        # one fewer exp instruction than the naive grouping
        idx = [(0,), (1, 7), (2, 6), (3, 5), (4,)]
    else:
        idx = ([(k,) for k in range(8)] + [(12,), (8,)]
               + [(9, 15), (10, 14), (11, 13)])
    return [[ent(kt) for kt in g] for g in idx]


def last_groups():
    """Final half keeps low-kt-only trailing groups so cols [0:512) can
    flush early (kt>11 touch only cols >= 512); (14) and (15) stay separate
    so cols [512:896) flush while the tiny kt15 tail finishes."""
    idx = [(k,) for k in range(12)] + [(12, 13), (14, 15)]
    Q0 = QH_W
    return [[(kt, max(Q0, 128 * kt), Q0 + QH_W) for kt in g] for g in idx]


def h0q0_groups():
    """First half: split so the first scores only need Q[:, 0:512]."""
    return [
        [(0, 0, 512)],
        [(1, 128, 512), (2, 256, 512), (3, 384, 512)],
        [(0, 512, 1024)],
        [(1, 512, 1024), (2, 512, 1024)],
        [(3, 512, 1024), (4, 512, 1024)],
        [(5, 640, 1024), (6, 768, 1024), (7, 896, 1024)],
    ]


def build_nc():
    nc = bacc.Bacc("TRN2", target_bir_lowering=False)

    qT = nc.declare_dram_parameter("qT", [D, S], BF16, isOutput=False)
    kTd = nc.declare_dram_parameter("kTd", [D, S], BF16, isOutput=False)
    vT = nc.declare_dram_parameter("vT", [D, S], BF16, isOutput=False)
    wqT = nc.declare_dram_parameter("wqT", [D, DSL], BF16, isOutput=False)
    wkT = nc.declare_dram_parameter("wkT", [D, DSL], BF16, isOutput=False)
    wvT = nc.declare_dram_parameter("wvT", [D, DSL], BF16, isOutput=False)
    # packed small tensors: [0:2]=bq, [2:4]=bk, [4:132]=bvb (bf16 bits),
    # [132:196]=mask (bf16 bits)
    smallp = nc.declare_dram_parameter("smallp", [128, 196], F32, isOutput=False)
    # rows [65h, 65h+64) = unnormalized out^T for head h; row 65h+64 = denom
    out_t = nc.declare_dram_parameter(
        "out_t", [HPC * (HD + 1), S], F32, isOutput=True
    )

    with TileContext(nc) as tc:
        with tc.tile_pool(name="const", bufs=1) as cpool:
            qT_sb = cpool.tile([128, 4, S], BF16, tag="qT_sb")
            kT_sb = cpool.tile([128, 4, S], BF16, tag="kT_sb")
            vT_sb = cpool.tile([128, 4, S], BF16, tag="vT_sb")
            wq_sb = cpool.tile([128, 4, DSL], BF16, tag="wq_sb")
            wk_sb = cpool.tile([128, 4, DSL], BF16, tag="wk_sb")
            wv_sb = cpool.tile([128, 4, DSL], BF16, tag="wv_sb")
            small_sb = cpool.tile([128, 196], F32, tag="small_sb")
            bq_sb = small_sb[:, 0:2]
            bk_sb = small_sb[:, 2:4]
            bvb_sb = small_sb[:, 4:132].bitcast(BF16)
            mask_sb = small_sb[:, 132:196].bitcast(BF16)
            # projected tensors: chunk dim = head pair (dout 128-chunk)
            QT_sb = cpool.tile([128, 2, S], BF16, tag="QT_sb")
            KT_sb = cpool.tile([128, 2, S], BF16, tag="KT_sb")
            # V with ones column: [k-part, head, k-tile, dv+1]
            vaug_sb = cpool.tile([128, HPC, 16, HD + 1], BF16, tag="vaug_sb")

            # only the ones-column needs init; cols 0..63 are written by proj_v
            nc.vector.memset(vaug_sb[:, :, :, HD : HD + 1], 1.0)
            # warmup scratch: the PE clock ramps with continuous busy time
            # (0.65->1.2->2.4 GHz over ~3us), so spin the PE on throwaway
            # matmuls while the first DMAs are in flight
            warm_sb = cpool.tile([128, 256], BF16, tag="warm_sb")
            nc.vector.memset(warm_sb[:], 1.0)

            def load_w_dc(w_sb, wsrc, dc0, dc1):
                if dc1 - dc0 == 1:
                    nc.sync.dma_start(
                        w_sb[:, dc0, :], wsrc[128 * dc0 : 128 * dc0 + 128, :]
                    )
                else:
                    nc.sync.dma_start(
                        w_sb[:, dc0:dc1, :],
                        wsrc[128 * dc0 : 128 * dc1, :].rearrange(
                            "(c p) m -> p c m", p=128
                        ),
                    )

            def load_x(dstt, srcd, s0, s1):
                nc.sync.dma_start(
                    dstt[:, :, s0:s1],
                    srcd[:, s0:s1].rearrange("(c p) s -> p c s", p=128),
                )

            # DMA order: prioritize the K->Q scores chain, V next, big tails
            # last.  Few large copies: every DMA pays a ~0.65us HWDGE slot,
            # so splitting loads only delays the chain's tail.
            load_w_dc(wk_sb, wkT, 0, 4)
            load_x(kT_sb, kTd, 0, 512)
            nc.sync.dma_start(small_sb[:], smallp[:])
            load_w_dc(wq_sb, wqT, 0, 4)
            load_x(qT_sb, qT, 0, 512)
            load_x(qT_sb, qT, 512, 1024)
            load_x(kT_sb, kTd, 512, 1024)
            load_w_dc(wv_sb, wvT, 0, 4)
            load_x(vT_sb, vT, 0, 512)
            load_x(vT_sb, vT, 512, 1024)
            load_x(qT_sb, qT, 1024, 2048)
            load_x(kT_sb, kTd, 1024, 2048)
            load_x(vT_sb, vT, 1024, 2048)

            # ---- projections + attention, flat pipeline ----
            # PSUM budget: ppool 2x1 + spool 2x2 + apool 1x2 = 8 banks
            with (
                tc.tile_pool(name="ppsum", bufs=2, space="PSUM") as ppool,
                tc.tile_pool(name="spsum", bufs=2, space="PSUM") as spool,
                tc.tile_pool(name="apsum", bufs=1, space="PSUM") as apool,
                tc.tile_pool(name="epool", bufs=12) as epool,
                tc.tile_pool(name="opool", bufs=6) as opool,
            ):

                def proj_v(st, h0, nh):
                    """V projection for seq window st, heads [h0, h0+nh)."""
                    c0, w = HD * h0, HD * nh
                    ps = ppool.tile([128, 512], F32, tag="pproj", name="psv")
                    for dc in range(4):
                        nc.tensor.matmul(
                            ps[:, 0:w],
                            vT_sb[:, dc, 128 * st : 128 * st + 128],
                            wv_sb[:, dc, c0 : c0 + w],
                            start=(dc == 0),
                            stop=(dc == 3),
                        )
                    nc.vector.tensor_add(
                        vaug_sb[:, h0 : h0 + nh, st, 0:HD],
                        ps[:, 0:w].rearrange("p (h d) -> p h d", h=nh),
                        bvb_sb[:, c0 : c0 + w].rearrange("p (h d) -> p h d", h=nh),
                    )

                QSRC = (wq_sb, bq_sb, qT_sb, QT_sb)
                KSRC = (wk_sb, bk_sb, kT_sb, KT_sb)

                def proj_qk_tile(mc, sc, src):
                    w_sb, b_sb, x_sb, dst = src
                    ps = ppool.tile([128, 512], F32, tag="pproj", name="psqk")
                    for dc in range(4):
                        nc.tensor.matmul(
                            ps[:],
                            w_sb[:, dc, 128 * mc : 128 * mc + 128],
                            x_sb[:, dc, 512 * sc : 512 * sc + 512],
                            start=(dc == 0),
                            stop=(dc == 3),
                        )
                    nc.vector.tensor_scalar_add(
                        dst[:, mc, 512 * sc : 512 * sc + 512],
                        ps[:],
                        b_sb[:, mc : mc + 1],
                    )

                def grp_offsets(grp):
                    # pack members tightly; a scores region must not
                    # cross a 512-element PSUM bank boundary
                    pos, offs = 0, []
                    for kt, a, b in grp:
                        W = b - a
                        if pos % 512 + min(W, 512) > 512:
                            pos = (pos + 511) // 512 * 512
                        offs.append(pos)
                        pos += W
                    return offs, pos

                def scores_grp(h, grp):
                    mc, prow = h // 2, 64 * (h % 2)
                    sl = spool.tile([128, QH_W], F32, tag="sl", name="sl")
                    offs = grp_offsets(grp)[0]
                    for (kt, a, b), base in zip(grp, offs):
                        K0, W = 128 * kt, b - a
                        for c0 in range(0, W, 512):
                            cw = min(512, W - c0)
                            nc.tensor.matmul(
                                sl[:, base + c0 : base + c0 + cw],
                                KT_sb[prow : prow + 64, mc, K0 : K0 + 128],
                                QT_sb[prow : prow + 64, mc, a + c0 : a + c0 + cw],
                                start=True,
                                stop=True,
                            )
                    return sl

                # ---- flat schedule over (head, q-half) pairs ----
                HALVES = [(0, 0), (1, 0), (0, 1), (1, 1),
                          (2, 0), (3, 0), (2, 1), (3, 1)]

                def qk(mc, sc, s):
                    return lambda: proj_qk_tile(mc, sc, s)

                def pv(st, h0, nh):
                    return lambda: proj_v(st, h0, nh)

                def make_scheds():
                    # per-half dict: group index -> ("slot", [thunks]) where
                    # slot M = after the scores lookahead, before this
                    # group's PV (PE filler while ACT runs exp); slot T =
                    # after this group's PV (for tiles whose input DMA lands
                    # later — keeps them off the PV critical path)
                    M, T = "M", "T"
                    return {
                        (0, 0): {
                            0: [(M, qk(0, 1, QSRC)), (T, qk(1, 0, QSRC))],
                            2: [(M, qk(0, 1, KSRC))],
                            3: [(M, qk(1, 1, QSRC))],
                            4: [(M, pv(0, 0, 2)), (M, pv(1, 0, 2)),
                                (M, pv(2, 0, 2)), (M, pv(3, 0, 2))],
                            5: [(M, pv(4, 0, 2)), (M, pv(5, 0, 2)),
                                (M, qk(1, 1, KSRC))],
                        },
                        (1, 0): {
                            0: [(M, pv(6, 0, 2)), (M, pv(7, 0, 2))],
                            2: [(M, qk(0, 2, QSRC))],
                            3: [(M, qk(0, 3, QSRC))],
                        },
                        (0, 1): {
                            0: [(M, qk(1, 2, QSRC))],
                            2: [(M, qk(1, 3, QSRC))],
                            5: [(M, qk(0, 2, KSRC)), (M, pv(8, 0, 2))],
                            6: [(M, qk(0, 3, KSRC)), (M, pv(9, 0, 2))],
                            7: [(M, pv(10, 0, 2))],
                            8: [(M, pv(11, 0, 2))],
                            9: [(M, pv(12, 0, 2))],
                            10: [(M, pv(13, 0, 2))],
                            11: [(M, pv(14, 0, 2))],
                            12: [(M, pv(15, 0, 2))],
                        },
                        (1, 1): {
                            0: [(M, qk(1, 2, KSRC))],
                            4: [(M, qk(1, 3, KSRC))],
                        },
                        (2, 0): {
                            0: [(M, pv(0, 2, 2))],
                            1: [(M, pv(1, 2, 2)), (M, pv(2, 2, 2))],
                            2: [(M, pv(3, 2, 2)), (M, pv(4, 2, 2))],
                            3: [(M, pv(5, 2, 2)), (M, pv(6, 2, 2))],
                            4: [(M, pv(7, 2, 2))],
                        },
                        (3, 0): {
                            0: [(M, pv(8, 2, 1))],
                            1: [(M, pv(9, 2, 1)), (M, pv(10, 2, 1))],
                            2: [(M, pv(11, 2, 1)), (M, pv(8, 3, 1))],
                            3: [(M, pv(9, 3, 1)), (M, pv(10, 3, 1))],
                            4: [(M, pv(11, 3, 1))],
                        },
                        (2, 1): {
                            8: [(M, pv(12, 2, 1))],
                            9: [(M, pv(13, 2, 1))],
                            10: [(M, pv(14, 2, 1))],
                            11: [(M, pv(15, 2, 1))],
                        },
                        (3, 1): {
                            8: [(M, pv(12, 3, 1))],
                            9: [(M, pv(13, 3, 1))],
                            10: [(M, pv(14, 3, 1))],
                            11: [(M, pv(15, 3, 1))],
                        },
                    }

                def flush(h, qh, acc, c0, c1):
                    # acc is (accA, accB): separate 1-bank tiles for q-cols
                    # [0:512) and [512:1024) so flush copies never couple
                    # with PV writes of the other half through tile deps
                    ot = opool.tile([HD + 1, 512], F32, tag="ot", name="ot")
                    t = acc[c0 // 512]
                    nc.vector.tensor_copy(ot[:, 0 : c1 - c0],
                                          t[:, c0 % 512 : c0 % 512 + c1 - c0])
                    r0 = (HD + 1) * h
                    Q0 = QH_W * qh
                    nc.sync.dma_start(
                        out_t[r0 : r0 + HD + 1, Q0 + c0 : Q0 + c1],
                        ot[:, 0 : c1 - c0],
                    )

                def emit_pv(rec):
                    # PV matmuls for a group, several pipeline stages behind
                    # the exp: everything here is ready, so the PE never
                    # blocks
                    h, qh, grp, goffs, et, acc, post = rec
                    Q0 = QH_W * qh
                    kmax = 8 if qh == 0 else 16
                    for (kt, a, b), base in zip(grp, goffs):
                        off = a - Q0
                        b0 = off
                        while b0 < off + (b - a):
                            b1 = min(off + (b - a), (b0 // 512 + 1) * 512)
                            nc.tensor.matmul(
                                acc[b0 // 512][:, b0 % 512 : b0 % 512 + b1 - b0],
                                vaug_sb[:, h, kt, :],
                                et[:, base + b0 - off : base + b1 - off],
                                start=(kt == 0),
                                stop=(kt == kmax - 1),
                                skip_group_check=True,
                            )
                            b0 = b1
                    for fl in post:
                        fl()

                for _pass in range(PASSES):
                    scheds = make_scheds()
                    groups_of = {
                        hq: (h0q0_groups() if hq == (0, 0)
                             else last_groups() if hq == HALVES[-1]
                             else std_groups(hq[1]))
                        for hq in HALVES
                    }
                    # prologue: PE warmup spin (clock ramp) while DMAs land,
                    # then the projections feeding the very first scores;
                    # K(mc1) last so its bias-add doesn't gate the Q tile
                    # through the ppool rotation
                    wps = ppool.tile([128, 512], F32, tag="pproj", name="warm")
                    for _w in range(14):
                        nc.tensor.matmul(
                            wps[0:1, 0:256], warm_sb[:, 0:1], warm_sb[:],
                            start=True, stop=True,
                        )
                    proj_qk_tile(0, 0, KSRC)
                    proj_qk_tile(0, 0, QSRC)
                    proj_qk_tile(1, 0, KSRC)

                    pending_sl = scores_grp(HALVES[0][0], groups_of[HALVES[0]][0])
                    stash_sl = None
                    pvq = []  # deferred PV records (deeper lag at startup
                    #           so early PVs don't block the PE stream while
                    #           the V DMAs are still in flight)
                    for hi, (h, qh) in enumerate(HALVES):
                        groups = groups_of[(h, qh)]
                        sched = scheds[(h, qh)]
                        acc = (apool.tile([HD + 1, 512], F32, tag="accA",
                                          name="accA"),
                               apool.tile([HD + 1, 512], F32, tag="accB",
                                          name="accB"))
                        last_half = hi + 1 >= len(HALVES)
                        for gi, grp in enumerate(groups):
                            sl = pending_sl
                            goffs, We = grp_offsets(grp)
                            et = epool.tile([128, QH_W], BF16, tag="et",
                                            name="et")
                            nc.scalar.activation(
                                et[:, 0:We], sl[:, 0:We], AF.Exp,
                                scale=1.0 / SCALE,
                            )
                            for (kt, a, b), base in zip(grp, goffs):
                                if a == 128 * kt:  # diagonal: causal mask
                                    nc.vector.tensor_mul(
                                        et[:, base : base + 128],
                                        et[:, base : base + 128],
                                        mask_sb[:],
                                    )
                            # one-group scores lookahead (crosses halves)
                            if gi + 1 < len(groups):
                                pending_sl = scores_grp(h, groups[gi + 1])
                            elif not last_half:
                                if stash_sl is not None:
                                    pending_sl = stash_sl
                                    stash_sl = None
                                else:
                                    nh, nqh = HALVES[hi + 1]
                                    pending_sl = scores_grp(
                                        nh, groups_of[(nh, nqh)][0]
                                    )
                            else:
                                pending_sl = None
                            for slot, work in sched.get(gi, ()):
                                if slot == "M":
                                    work()
                            if hi == 0 and gi < 4:
                                lag = 99
                            elif last_half and gi >= 8:
                                lag = max(1, 13 - gi)
                            else:
                                lag = 9
                            while len(pvq) > lag:
                                emit_pv(pvq.pop(0))
                            for slot, work in sched.get(gi, ()):
                                if slot == "T":
                                    work()
                            # flushes ride on the PV record (run after it);
                            # the final half streams out in 4 chunks, each as
                            # soon as its last k-tile has accumulated
                            post = []

                            def fl(c0, c1, h_=h, q_=qh, a_=acc):
                                return lambda: flush(h_, q_, a_, c0, c1)

                            # cols [0:512) finish early (only low k-tiles
                            # touch them): flush A as soon as its last
                            # contributor's PV lands, B at the half's end
                            if hi == 0:
                                a_gi = 1
                            elif qh == 0:
                                a_gi = 3
                            else:
                                a_gi = len(groups) - 1
                            if gi == a_gi:
                                post.append(fl(0, 512))
                            if gi + 1 == len(groups):
                                post.append(fl(512, QH_W))
                            pvq.append((h, qh, grp, goffs, et, acc, post))
                            if (gi + 2 == len(groups) and not last_half
                                    and hi > 0):
                                # boundary smoothing: emit the next half's
                                # first scores one iteration early (PE
                                # head-blocks on the spool buffer only until
                                # the exp two groups back retires)
                                nh, nqh = HALVES[hi + 1]
                                stash_sl = scores_grp(
                                    nh, groups_of[(nh, nqh)][0]
                                )
                    # drain the final PVs after the last exp
                    for rec in pvq:
                        emit_pv(rec)

    nc.finalize()
    return nc


_NC_CACHE = {}


def _get_nc():
    if "nc" not in _NC_CACHE:
        _NC_CACHE["nc"] = build_nc()
    return _NC_CACHE["nc"]


def make_in_maps(query, key, value, Wq, bq, Wk, bk, Wv, bv):
    query, key, value = (np.asarray(x, np.float32) for x in (query, key, value))
    Wq, Wk, Wv = (np.asarray(x, np.float32) for x in (Wq, Wk, Wv))
    bq, bk, bv = (np.asarray(x, np.float32) for x in (bq, bk, bv))
    mask = np.triu(np.ones((128, 128), np.float32)).astype(BFNP)

    def pack_small(bqs, bks, bvs, m):
        out = np.empty((128, 196), np.float32)
        out[:, 0:2] = bqs.reshape(2, 128).T
        out[:, 2:4] = bks.reshape(2, 128).T
        bvt = np.tile(bvs[None, :].astype(BFNP), (128, 1))
        out[:, 4:132] = np.ascontiguousarray(bvt).view(np.float32)
        out[:, 132:196] = np.ascontiguousarray(m).view(np.float32)
        return out

    in_maps = []
    for c in range(N_CORES):
        b, g = c // 2, c % 2
        sl = slice(DSL * g, DSL * g + DSL)
        in_maps.append(
            {
                "qT": np.ascontiguousarray(query[b].astype(BFNP).T),
                "kTd": np.ascontiguousarray(key[b].astype(BFNP).T),
                "vT": np.ascontiguousarray(value[b].astype(BFNP).T),
                "wqT": np.ascontiguousarray(Wq[sl].astype(BFNP).T),
                "wkT": np.ascontiguousarray(Wk[sl].astype(BFNP).T),
                "wvT": np.ascontiguousarray(Wv[sl].astype(BFNP).T),
                "smallp": pack_small(bq[sl], bk[sl], bv[sl], mask),
            }
        )
    return in_maps


def assemble_output(results):
    out = np.empty((B, S, D), np.float32)
    for c in range(N_CORES):
        b, g = c // 2, c % 2
        ot = results[c]["out_t"]  # [260, 2048]
        for hl in range(HPC):
            blk = ot[(HD + 1) * hl : (HD + 1) * hl + HD]  # [64, S]
            den = ot[(HD + 1) * hl + HD]  # [S]
            h = HPC * g + hl
            out[b, :, HD * h : HD * h + HD] = (blk / den).T
    return out


def run(trace=False, **inputs):
    nc = _get_nc()
    in_maps = make_in_maps(**inputs)
    res = run_bass_kernel_spmd(nc, in_maps, list(range(N_CORES)), trace=trace)
    return assemble_output(res.results), res


def kernel(**inputs) -> np.ndarray:
    out, _ = run(trace=False, **inputs)
    return out
